# revision 18
# baseline (speedup 1.0000x reference)
"""Trainium2 Bass kernel for nn_Communication (gnn_message_passing).

Sharding: H=128 rows split 16/core across 8 NeuronCores; every core handles all
N=5 batch elements and all C=256 channels for its row slab. Each core receives
a 20-row slice (2-row halos, zero rows at the global image edges) so the SPMD
program is identical on every core. The only cross-core communication is one
AllGather of the per-core [128,20] channel-pool partials (avg-sums and maxes);
the scalar loss/mean-rate reductions return per-core partials that the host
sums.

Precision: attention/mask path in fp32; the statistics network (two 3-layer
pointwise MLPs, ~87% of FLOPs) runs bf16 on the TensorEngine with fp32 PSUM
accumulation. T and T' share feat/sparse bf16 tiles per (n, s-tile).
"""
import numpy as np
import ml_dtypes

import concourse.bass as bass
import concourse.tile as tile
import concourse.mybir as mybir
from concourse import bacc
from concourse.bass_utils import run_bass_kernel_spmd
from concourse.masks import make_identity

NCORES = 8
N, C, H, W = 5, 256, 128, 128
R = H // NCORES          # owned rows per core (16)
RH = R + 4               # rows incl 2-row halo (20)
KC = C // 128            # channel chunks (2)
OB = 8                   # 1024 hidden / 128
TS = 512                 # free-dim tile (4 rows x 128)
NT = (R * W) // TS       # s-tiles per core (4)
SFULL = RH * W           # 2560 (free size incl halo)
OWN0 = 2 * W             # first owned col (rows 2..18 of 20)
THRE = 0.01
F32 = mybir.dt.float32
BF16 = mybir.dt.bfloat16
AX = mybir.AxisListType
OP = mybir.AluOpType
AF = mybir.ActivationFunctionType

# gaussian taps (compile-time constants, replicated from reference)
_c = 3 // 2
_x, _y = np.mgrid[-_c:3 - _c, -_c:3 - _c]
G2 = (1.0 / (2.0 * np.pi) * np.exp(-(_x ** 2 + _y ** 2) / 2.0)).astype(np.float32)
_g1x = np.arange(-1, 2, dtype=np.float32)
_g1 = np.exp(-_g1x ** 2 / 2.0)
G1 = (_g1 / _g1.sum()).astype(np.float32)

# consts vector indices
CI_SPW0 = 0     # 9 taps, in-ch 0 (pre-scaled by 1/256)
CI_SPW1 = 9     # 9 taps, in-ch 1
CI_WA = 18
CI_NEGWA = 19
CI_WB = 20
CI_B3 = 21
CI_NEGB3 = 22
NCONST = 24

_CACHED = {}


def _bf(ap2d, nrep):
    """Free-dim broadcast: [P,1] AP -> [P,nrep] via 0-stride."""
    return bass.AP(ap2d.tensor, ap2d.offset, [list(ap2d.ap[0]), [0, nrep]])


ACT_TABLE_PATCH = True


def _patch_act_tables():
    """Order activation tables so one table (Relu+Exp+Ln+Copy) serves the
    whole main loop -- avoids per-tile ACT table reloads."""
    import concourse.bacc as _bm
    if not ACT_TABLE_PATCH or getattr(_bm, "_act_tbl_patched", False):
        return
    _orig = _bm.get_activation_tables

    def _reordered(arch):
        t = _orig(arch)
        pref = 'natural_log_exp_and_others'
        order = [pref] + [k for k in t if k != pref]
        return {k: t[k] for k in order}

    _bm.get_activation_tables = _reordered
    _bm._act_tbl_patched = True


def build():
    _patch_act_tables()
    nc = bacc.Bacc("TRN2", target_bir_lowering=False, debug=False,
                   num_devices=NCORES)
    feat = nc.dram_tensor("feat", [N, C, RH, W], F32, kind="ExternalInput").ap()
    w1t = nc.dram_tensor("w1t", [2 * C, 1024], BF16, kind="ExternalInput").ap()
    w2t = nc.dram_tensor("w2t", [1024, 1024], BF16, kind="ExternalInput").ap()
    w3t = nc.dram_tensor("w3t", [1024, 1], BF16, kind="ExternalInput").ap()
    b1m = nc.dram_tensor("b1m", [128, OB], F32, kind="ExternalInput").ap()
    b2m = nc.dram_tensor("b2m", [128, OB], F32, kind="ExternalInput").ap()
    m1t = nc.dram_tensor("m1t", [C, 16], F32, kind="ExternalInput").ap()
    m2t = nc.dram_tensor("m2t", [16, C], F32, kind="ExternalInput").ap()
    wat = nc.dram_tensor("wat", [C, C], F32, kind="ExternalInput").ap()
    wbt = nc.dram_tensor("wbt", [C, C], F32, kind="ExternalInput").ap()
    cst = nc.dram_tensor("cst", [NCONST], F32, kind="ExternalInput").ap()
    vld = nc.dram_tensor("vld", [RH, 1], F32, kind="ExternalInput").ap()

    sf = nc.dram_tensor("sf", [N, C, R, W], F32, kind="ExternalOutput").ap()
    sm = nc.dram_tensor("sm", [N, C, R, W], F32, kind="ExternalOutput").ap()
    stats = nc.dram_tensor("stats", [1, 4], F32, kind="ExternalOutput").ap()

    with tile.TileContext(nc) as tc:
        _build_body(nc, tc, feat, w1t, w2t, w3t, b1m, b2m, m1t, m2t, wat, wbt,
                    cst, vld, sf, sm, stats)
    nc.compile()
    return nc




def _build_body(nc, tc, feat, w1t, w2t, w3t, b1m, b2m, m1t, m2t, wat, wbt,
                cst, vld, sf, sm, stats):
    f32, bf16 = F32, BF16
    respool = tc.tile_pool(name="res", bufs=1)
    res = respool.__enter__()

    def _tc_tile(tc, shape, dtype, name):
        return res.tile(shape, dtype, tag=name, name=name)

    # ---------------- persistent tiles ----------------
    ft = _tc_tile(tc, [128, 2 * N, SFULL], f32, name="ft")          # resident feat
    w1s = _tc_tile(tc, [128, 4, 1024], bf16, name="w1s")
    w2s = _tc_tile(tc, [128, 8, 1024], bf16, name="w2s")
    w3s = _tc_tile(tc, [128, OB, 1], bf16, name="w3s")
    b1s = _tc_tile(tc, [128, OB], f32, name="b1s")
    b2s = _tc_tile(tc, [128, OB], f32, name="b2s")
    m1s = _tc_tile(tc, [128, KC, 16], f32, name="m1s")
    m2s = _tc_tile(tc, [16, C], f32, name="m2s")
    was = _tc_tile(tc, [128, KC, C], f32, name="was")
    wbs = _tc_tile(tc, [128, KC, C], f32, name="wbs")
    cs = _tc_tile(tc, [128, NCONST], f32, name="cs")
    vst = _tc_tile(tc, [120, 1], f32, name="vst")
    ones = _tc_tile(tc, [128, TS], f32, name="ones")
    id4 = _tc_tile(tc, [4, 4], f32, name="id4")

    # conv buffers: 5 row-blocks of 24 partitions; data rows r=0..19 at
    # partition 24n+r, cols 1..128 hold W, col 0/129 stay zero.
    CW = 132
    cmsum = _tc_tile(tc, [120, CW], f32, name="cmsum")
    cmmax = _tc_tile(tc, [120, CW], f32, name="cmmax")
    sigm = _tc_tile(tc, [120, CW], f32, name="sigm")
    spatt = _tc_tile(tc, [120, CW], f32, name="spatt")
    ebuf = _tc_tile(tc, [24, CW], f32, name="ebuf")
    spcin = _tc_tile(tc, [120, CW], f32, name="spcin")
    spcoef = _tc_tile(tc, [120, CW], f32, name="spcoef")
    actm = _tc_tile(tc, [120, CW], f32, name="actm")
    spact = _tc_tile(tc, [120, CW], f32, name="spact")

    pool_p = _tc_tile(tc, [128, 20], f32, name="pool_p")            # local partials
    gsum = _tc_tile(tc, [128, 10], f32, name="gsum")
    gmax = _tc_tile(tc, [128, 10], f32, name="gmax")
    avs = _tc_tile(tc, [128, 10], f32, name="avs")
    chatt = _tc_tile(tc, [128, 10], f32, name="chatt")
    ego4 = _tc_tile(tc, [128, KC], f32, name="ego4")
    ego4m = _tc_tile(tc, [128, 4], f32, name="ego4m")
    chpad = _tc_tile(tc, [4, C + 2], f32, name="chpad")
    chc4 = _tc_tile(tc, [4, C], f32, name="chc4")
    chcoef = _tc_tile(tc, [128, 2 * 4], f32, name="chcoef")         # col kc*4+m
    jacc = _tc_tile(tc, [1, N * NT], f32, name="jacc")
    macc = _tc_tile(tc, [1, N * NT], f32, name="macc")
    mks = _tc_tile(tc, [128, 4 * KC * NT], f32, name="mks")
    statsb = _tc_tile(tc, [1, 4], f32, name="statsb")
    spscr = _tc_tile(tc, [1, TS], f32, name="spscr")
    spscr2 = _tc_tile(tc, [1, TS], f32, name="spscr2")

    for t_ in (cmsum, cmmax, sigm, spatt, spcin, spcoef, actm, spact, ebuf,
               vst, statsb):
        nc.vector.memset(t_, 0.0)
    nc.vector.memset(ones, 1.0)
    make_identity(nc, id4)

    # ---------------- input DMAs ----------------
    for n in range(N):
        for kc in range(KC):
            nc.sync.dma_start(
                out=ft[:, n * KC + kc, :],
                in_=feat[n, kc * 128:(kc + 1) * 128, :, :])
    nc.sync.dma_start(out=w1s, in_=bass.AP(
        w1t.tensor, 0, [[1024, 128], [128 * 1024, 4], [1, 1024]]))
    nc.sync.dma_start(out=w2s, in_=bass.AP(
        w2t.tensor, 0, [[1024, 128], [128 * 1024, 8], [1, 1024]]))
    nc.sync.dma_start(out=w3s, in_=bass.AP(
        w3t.tensor, 0, [[1, 128], [128, OB], [1, 1]]))
    nc.sync.dma_start(out=b1s, in_=b1m)
    nc.sync.dma_start(out=b2s, in_=b2m)
    nc.sync.dma_start(out=m1s, in_=bass.AP(
        m1t.tensor, 0, [[16, 128], [128 * 16, KC], [1, 16]]))
    nc.sync.dma_start(out=m2s, in_=m2t)
    nc.sync.dma_start(out=was, in_=bass.AP(
        wat.tensor, 0, [[C, 128], [128 * C, KC], [1, C]]))
    nc.sync.dma_start(out=wbs, in_=bass.AP(
        wbt.tensor, 0, [[C, 128], [128 * C, KC], [1, C]]))
    nc.sync.dma_start(out=cs, in_=bass.AP(
        cst.tensor, 0, [[0, 128], [1, NCONST]]))
    for n in range(N):
        nc.sync.dma_start(out=vst[24 * n:24 * n + RH, 0:1], in_=vld)

    def c1(j, lo, hi):
        return cs[lo:hi, j:j + 1]

    # ---------------- phase 0: pools + collective ----------------
    for n in range(N):
        for kc in range(KC):
            i = n * KC + kc
            nc.vector.reduce_sum(out=pool_p[:, kc * N + n: kc * N + n + 1],
                                 in_=ft[:, i, OWN0:OWN0 + R * W], axis=AX.X)
            nc.vector.tensor_reduce(out=pool_p[:, 10 + kc * N + n: 11 + kc * N + n],
                                    in_=ft[:, i, OWN0:OWN0 + R * W], axis=AX.X,
                                    op=OP.max)
    with tc.tile_pool(name="ccd", bufs=1, space="DRAM") as ccd:
        cc_in = ccd.tile([128, 20], f32)
        cc_out = ccd.tile([NCORES * 128, 20], f32, addr_space="Shared")
        nc.sync.dma_start(out=cc_in, in_=pool_p)
        nc.gpsimd.collective_compute(
            "AllGather", OP.bypass,
            replica_groups=[list(range(NCORES))],
            ins=[cc_in.opt()], outs=[cc_out.opt()])
        gath = _tc_tile(tc, [128, NCORES, 20], f32, name="gath")
        nc.sync.dma_start(out=gath, in_=bass.AP(
            cc_out.tensor, cc_out.offset, [[20, 128], [128 * 20, NCORES], [1, 20]]))
    # reduce over the 8 gathered shards
    nc.vector.reduce_sum(out=gsum, in_=bass.AP(
        gath.tensor, gath.offset, [list(gath.ap[0]), [1, 10], [20, NCORES]]),
        axis=AX.X)
    nc.vector.tensor_reduce(out=gmax, in_=bass.AP(
        gath.tensor, gath.offset + 10, [list(gath.ap[0]), [1, 10], [20, NCORES]]),
        axis=AX.X, op=OP.max)

    # ---------------- phase 0b: channel maps + convs (fp32, DVE) ----------
    from concourse import bass_isa
    with tc.tile_pool(name="tree", bufs=1) as tree, \
         tc.tile_pool(name="csps", bufs=2, space="PSUM") as csps:
        for n in range(N):
            # channel sums on the (otherwise idle) TensorEngine
            csrow = tree.tile([1, SFULL], f32, tag="csrow")
            for ch in range(SFULL // TS):
                cps = csps.tile([1, TS], f32, tag="csum")
                for kc in range(KC):
                    nc.tensor.matmul(cps, ones[:, 0:1],
                                     ft[:, n * KC + kc, ch * TS:(ch + 1) * TS],
                                     start=(kc == 0), stop=(kc == 1))
                nc.scalar.activation(out=csrow[0:1, ch * TS:(ch + 1) * TS],
                                     in_=cps, func=AF.Copy)
            nc.sync.dma_start(
                out=cmsum[24 * n:24 * n + RH, 1:129],
                in_=csrow[0:1, 0:SFULL])
            # channel maxes via gpsimd partition all-reduce
            tmp = tree.tile([128, SFULL], f32, tag="tmp")
            nc.vector.tensor_tensor(out=tmp, in0=ft[:, n * KC, :],
                                    in1=ft[:, n * KC + 1, :], op=OP.max)
            nc.gpsimd.partition_all_reduce(out_ap=tmp[:], in_ap=tmp[:],
                                           channels=128,
                                           reduce_op=bass_isa.ReduceOp.max)
            nc.sync.dma_start(
                out=cmmax[24 * n:24 * n + RH, 1:129],
                in_=tmp[0:1, 0:SFULL])

    cscr = _tc_tile(tc, [120, CW], f32, name="cscr")
    nc.vector.memset(cscr, 0.0)

    def conv3(dst, srcs_taps):
        """dst[0:120,1:129] = sum over (src, taps9): taps[3dy+dx]*src[p+dy-1, dx:dx+128].
        Row (partition) shifts go through a DMA copy; every compute op is
        base-partition-0 over all 120 rows (junk rows are never consumed)."""
        first = True
        for src, taps in srcs_taps:
            for dy in range(3):
                if dy == 1:
                    sb = src
                else:
                    if dy == 0:
                        nc.sync.dma_start(out=cscr[1:120, :], in_=src[0:119, :])
                    else:
                        nc.sync.dma_start(out=cscr[0:119, :], in_=src[1:120, :])
                    sb = cscr
                for dx in range(3):
                    tap = taps[3 * dy + dx]
                    in0 = sb[0:120, dx:dx + 128]
                    o = dst[0:120, 1:129]
                    if isinstance(tap, (float, int)):
                        s = float(tap)
                    else:
                        s = c1(tap, 0, 120)
                    if first:
                        nc.vector.tensor_scalar(out=o, in0=in0, scalar1=s,
                                                scalar2=None, op0=OP.mult)
                        first = False
                    else:
                        nc.vector.scalar_tensor_tensor(
                            out=o, in0=in0, scalar=s, in1=o,
                            op0=OP.mult, op1=OP.add)

    # sp_att = sigmoid(conv([mean, max]))
    conv3(spatt, [(cmsum, [CI_SPW0 + j for j in range(9)]),
                  (cmmax, [CI_SPW1 + j for j in range(9)])])
    nc.scalar.activation(out=spatt[0:120, 1:129], in_=spatt[0:120, 1:129],
                         func=AF.Sigmoid)
    # act = conv(sigmoid(mean/256) * valid, g2)
    nc.scalar.activation(out=sigm[0:120, 1:129], in_=cmsum[0:120, 1:129],
                         func=AF.Sigmoid, scale=1.0 / 256.0)
    nc.vector.tensor_scalar(out=sigm[0:120, 1:129], in0=sigm[0:120, 1:129],
                            scalar1=vst[0:120, 0:1], scalar2=None, op0=OP.mult)
    conv3(actm, [(sigm, [float(G2[dy, dx]) for dy in range(3) for dx in range(3)])])
    # sp_coef = conv(sigmoid(wa*ego + wb*sp_att[1:]) * valid, g2)
    nc.vector.tensor_scalar(out=ebuf[0:20, 1:129], in0=spatt[0:20, 1:129],
                            scalar1=c1(CI_NEGWA, 0, 20), scalar2=c1(CI_WA, 0, 20),
                            op0=OP.mult, op1=OP.add)
    for m in range(1, N):
        nc.sync.dma_start(out=spcin[24 * m:24 * m + 20, 1:129],
                          in_=ebuf[0:20, 1:129])
    nc.vector.scalar_tensor_tensor(
        out=spcin[0:120, 1:129], in0=spatt[0:120, 1:129],
        scalar=c1(CI_WB, 0, 120), in1=spcin[0:120, 1:129],
        op0=OP.mult, op1=OP.add)
    nc.scalar.activation(out=spcin[0:120, 1:129], in_=spcin[0:120, 1:129],
                         func=AF.Sigmoid)
    nc.vector.tensor_scalar(out=spcin[0:120, 1:129], in0=spcin[0:120, 1:129],
                            scalar1=vst[0:120, 0:1], scalar2=None, op0=OP.mult)
    conv3(spcoef, [(spcin, [float(G2[dy, dx]) for dy in range(3) for dx in range(3)])])
    nc.vector.tensor_tensor(out=spact[0:120, 1:129], in0=spcoef[0:120, 1:129],
                            in1=actm[0:120, 1:129], op=OP.mult)
    # flatten spact rows into free dim (single partition) for later broadcast
    spact_bf = _tc_tile(tc, [120, CW], bf16, name="spact_bf")
    nc.vector.tensor_copy(out=spact_bf[0:120, :], in_=spact[0:120, :])
    _saflcm = tc.tile_pool(name="safld", bufs=1, space="DRAM")
    saflpool = _saflcm.__enter__()
    safl = saflpool.tile([4, R * W], bf16)
    for m in range(4):
        nc.sync.dma_start(out=safl[m:m + 1, :],
                          in_=spact_bf[24 * (m + 1) + 2:24 * (m + 1) + 18, 1:129])

    # ---------------- phase 0c: MLP -> ch_att -> ch_coef ----------------
    with tc.tile_pool(name="p0ps", bufs=2, space="PSUM") as p0ps, \
         tc.tile_pool(name="p0sb", bufs=2) as p0sb:
        nc.scalar.mul(out=avs, in_=gsum, mul=1.0 / float(H * W))
        hidps = p0ps.tile([16, N], f32, tag="hid")
        for kc in range(KC):
            nc.tensor.matmul(hidps, m1s[:, kc, :], avs[:, kc * N:(kc + 1) * N],
                             start=(kc == 0), stop=(kc == 1))
        hida = p0sb.tile([16, N], f32, tag="hid_sb")
        nc.scalar.activation(out=hida, in_=hidps, func=AF.Relu)
        hidps2 = p0ps.tile([16, N], f32, tag="hid")
        for kc in range(KC):
            nc.tensor.matmul(hidps2, m1s[:, kc, :], gmax[:, kc * N:(kc + 1) * N],
                             start=(kc == 0), stop=(kc == 1))
        hidm = p0sb.tile([16, N], f32, tag="hid_sb")
        nc.scalar.activation(out=hidm, in_=hidps2, func=AF.Relu)
        for kco in range(KC):
            chps = p0ps.tile([128, N], f32, tag="chps")
            nc.tensor.matmul(chps, m2s[:, kco * 128:(kco + 1) * 128], hida,
                             start=True, stop=False)
            nc.tensor.matmul(chps, m2s[:, kco * 128:(kco + 1) * 128], hidm,
                             start=False, stop=True)
            nc.scalar.activation(out=chatt[:, kco * N:(kco + 1) * N], in_=chps,
                                 func=AF.Sigmoid)
        for kc in range(KC):
            nc.vector.tensor_scalar(out=ego4[:, kc:kc + 1],
                                    in0=chatt[:, kc * N:kc * N + 1],
                                    scalar1=-1.0, scalar2=1.0,
                                    op0=OP.mult, op1=OP.add)
        # ch_coef_pre[m, c] (m=0..3 -> n=1..4)
        chpre = p0ps.tile([4, C], f32, tag="chpre")
        for kc in range(KC):
            nc.vector.tensor_copy(out=ego4m[:, :], in_=_bf(ego4[:, kc:kc + 1], 4))
            nc.tensor.matmul(chpre, ego4m, was[:, kc, :],
                             start=(kc == 0), stop=False)
            nc.tensor.matmul(chpre, chatt[:, kc * N + 1:kc * N + N],
                             wbs[:, kc, :], start=False, stop=(kc == 1))
        nc.vector.memset(chpad, 0.0)
        nc.scalar.activation(out=chpad[:, 1:C + 1], in_=chpre, func=AF.Sigmoid)
        # 3-tap gaussian smoothing along C
        nc.vector.tensor_scalar(out=chc4, in0=chpad[:, 1:C + 1],
                                scalar1=float(G1[1]), scalar2=None, op0=OP.mult)
        nc.vector.scalar_tensor_tensor(out=chc4, in0=chpad[:, 0:C],
                                       scalar=float(G1[0]), in1=chc4,
                                       op0=OP.mult, op1=OP.add)
        nc.vector.scalar_tensor_tensor(out=chc4, in0=chpad[:, 2:C + 2],
                                       scalar=float(G1[2]), in1=chc4,
                                       op0=OP.mult, op1=OP.add)
        for kc in range(KC):
            tp = p0ps.tile([128, 4], f32, tag="tp")
            nc.tensor.transpose(tp, chc4[:, kc * 128:(kc + 1) * 128], id4)
            nc.scalar.activation(out=chcoef[:, kc * 4:(kc + 1) * 4], in_=tp,
                                 func=AF.Copy)

    # ---------------- main loop ----------------
    with tc.tile_pool(name="fbf", bufs=14) as fbfp, \
         tc.tile_pool(name="spb", bufs=3) as spbp, \
         tc.tile_pool(name="mstage", bufs=2) as mst, \
         tc.tile_pool(name="h1p", bufs=2) as h1p, \
         tc.tile_pool(name="h2p", bufs=2) as h2p, \
         tc.tile_pool(name="bcp", bufs=2) as bcp, \
         tc.tile_pool(name="psA", bufs=3, space="PSUM") as psA, \
         tc.tile_pool(name="psB", bufs=2, space="PSUM") as psB, \
         tc.tile_pool(name="psT", bufs=3, space="PSUM") as psT:

        fbf = {}

        def cast_feat(n, eng):
            for kc in range(KC):
                for t in range(NT):
                    tl = fbfp.tile([128, TS], bf16, tag="fbf")
                    eng.tensor_copy(
                        out=tl, in_=ft[:, n * KC + kc, OWN0 + t * TS:OWN0 + (t + 1) * TS])
                    fbf[(n, kc, t)] = tl

        cast_feat(0, nc.vector)
        cast_feat(1, nc.vector)

        for n in range(N):
            nxt = (n + 1) % N
            if 1 <= n <= N - 2:
                cast_feat(n + 1, nc.vector)
            elif n == N - 1:
                cast_feat(0, nc.vector)  # gen-0 tiles were recycled
            for t in range(NT):
                # ---- mask / sparse for this (n, t) ----
                if n == 0:
                    spbf = [fbf[(0, kc, t)] for kc in range(KC)]
                    for kc in range(KC):
                        nc.sync.dma_start(
                            out=sf[0, kc * 128:(kc + 1) * 128, 4 * t:4 * t + 4, :],
                            in_=ft[:, kc, OWN0 + t * TS:OWN0 + (t + 1) * TS])
                        nc.sync.dma_start(
                            out=sm[0, kc * 128:(kc + 1) * 128, 4 * t:4 * t + 4, :],
                            in_=ones)
                else:
                    m = n - 1
                    sab = bcp.tile([128, TS], bf16, tag="sab")
                    nc.sync.dma_start(out=sab, in_=bass.AP(
                        safl.tensor, safl.offset + m * (R * W) + t * TS,
                        [[0, 128], [1, TS]]))
                    spbf = []
                    for kc in range(KC):
                        mtile = mst.tile([128, TS], f32, tag="msk")
                        nc.vector.tensor_scalar(
                            out=mtile, in0=sab,
                            scalar1=chcoef[:, kc * 4 + m:kc * 4 + m + 1],
                            scalar2=THRE, op0=OP.mult, op1=OP.is_gt)
                        nc.vector.reduce_sum(
                            out=mks[:, m * KC * NT + kc * NT + t:
                                    m * KC * NT + kc * NT + t + 1],
                            in_=mtile, axis=AX.X)
                        nc.sync.dma_start(
                            out=sm[n, kc * 128:(kc + 1) * 128, 4 * t:4 * t + 4, :],
                            in_=mtile)
                        spf = mst.tile([128, TS], f32, tag="spf")
                        nc.vector.tensor_tensor(
                            out=spf, in0=mtile,
                            in1=ft[:, n * KC + kc, OWN0 + t * TS:OWN0 + (t + 1) * TS],
                            op=OP.mult)
                        nc.sync.dma_start(
                            out=sf[n, kc * 128:(kc + 1) * 128, 4 * t:4 * t + 4, :],
                            in_=spf)
                        sbf = spbp.tile([128, TS], bf16, tag="spb")
                        nc.vector.tensor_copy(out=sbf, in_=spf)
                        spbf.append(sbf)

                # ---- layer 1 (pair L=T[n], R=T'[n]) ----
                h1L = h1p.tile([128, OB, TS], bf16, tag="h1L")
                h1R = h1p.tile([128, OB, TS], bf16, tag="h1R")
                for ob in range(OB):
                    oc = slice(ob * 128, (ob + 1) * 128)
                    zL = psA.tile([128, TS], f32, tag="za")
                    zR = psA.tile([128, TS], f32, tag="za")
                    rhsL = [fbf[(n, 0, t)], fbf[(n, 1, t)], spbf[0], spbf[1]]
                    rhsR = [fbf[(nxt, 0, t)], fbf[(nxt, 1, t)], spbf[0], spbf[1]]
                    for r in range(4):
                        nc.tensor.matmul(zL, w1s[:, r, oc], rhsL[r],
                                         start=(r == 0), stop=(r == 3))
                        nc.tensor.matmul(zR, w1s[:, r, oc], rhsR[r],
                                         start=(r == 0), stop=(r == 3))
                    nc.scalar.activation(out=h1L[:, ob, :], in_=zL, func=AF.Relu,
                                         bias=b1s[:, ob:ob + 1])
                    nc.vector.tensor_scalar(out=h1R[:, ob, :], in0=zR,
                                            scalar1=b1s[:, ob:ob + 1],
                                            scalar2=0.0, op0=OP.add, op1=OP.max)

                # ---- layer 2 + layer 3 ----
                TL = psT.tile([1, TS], f32, tag="T")
                TR = psT.tile([1, TS], f32, tag="T")
                for ob in range(OB):
                    oc = slice(ob * 128, (ob + 1) * 128)
                    z2L = psB.tile([128, TS], f32, tag="zb")
                    z2R = psB.tile([128, TS], f32, tag="zb")
                    for kr in range(OB):
                        nc.tensor.matmul(z2L, w2s[:, kr, oc], h1L[:, kr, :],
                                         start=(kr == 0), stop=(kr == 7))
                        nc.tensor.matmul(z2R, w2s[:, kr, oc], h1R[:, kr, :],
                                         start=(kr == 0), stop=(kr == 7))
                    h2L = h2p.tile([128, TS], bf16, tag="h2L")
                    nc.scalar.activation(out=h2L, in_=z2L, func=AF.Relu,
                                         bias=b2s[:, ob:ob + 1])
                    h2R = h2p.tile([128, TS], bf16, tag="h2R")
                    nc.vector.tensor_scalar(out=h2R, in0=z2R,
                                            scalar1=b2s[:, ob:ob + 1],
                                            scalar2=0.0, op0=OP.add, op1=OP.max)
                    nc.tensor.matmul(TL, w3s[:, ob, :], h2L,
                                     start=(ob == 0), stop=(ob == 7))
                    nc.tensor.matmul(TR, w3s[:, ob, :], h2R,
                                     start=(ob == 0), stop=(ob == 7))
                # softplus partials
                idx = n * NT + t
                # softplus(x) = ln(1 + e^x); x = -(T+b3) for joint, +(T'+b3)
                nc.scalar.activation(out=spscr, in_=TL, func=AF.Exp,
                                     scale=-1.0, bias=c1(CI_NEGB3, 0, 1))
                nc.scalar.activation(out=spscr2, in_=TR, func=AF.Exp,
                                     scale=1.0, bias=c1(CI_B3, 0, 1))
                nc.scalar.activation(out=spscr, in_=spscr, func=AF.Ln,
                                     bias=1.0, accum_out=jacc[0:1, idx:idx + 1])
                nc.scalar.activation(out=spscr2, in_=spscr2, func=AF.Ln,
                                     bias=1.0, accum_out=macc[0:1, idx:idx + 1])

        # ---- final scalar partials ----
        nc.vector.reduce_sum(out=statsb[0:1, 0:1], in_=jacc, axis=AX.X)
        nc.vector.reduce_sum(out=statsb[0:1, 1:2], in_=macc, axis=AX.X)
        mkcol = _tc_tile(tc, [128, 1], f32, name="mkcol")
        nc.vector.reduce_sum(out=mkcol, in_=mks, axis=AX.X)
        stps = psT.tile([1, TS], f32, tag="T")
        nc.tensor.matmul(stps[0:1, 0:1], ones[:, 0:1], mkcol,
                         start=True, stop=True)
        nc.scalar.activation(out=statsb[0:1, 2:3], in_=stps[0:1, 0:1],
                             func=AF.Copy)
        nc.sync.dma_start(out=stats, in_=statsb)


def _prep_inputs(inputs):
    f32 = np.float32
    bf = ml_dtypes.bfloat16
    feat = np.ascontiguousarray(inputs["feat"], dtype=f32)
    padded = np.zeros((N, C, H + 4, W), f32)
    padded[:, :, 2:H + 2, :] = feat
    st_w1 = np.asarray(inputs["st_w1"], f32)
    st_w2 = np.asarray(inputs["st_w2"], f32)
    st_w3 = np.asarray(inputs["st_w3"], f32)
    ch_fus_w = np.asarray(inputs["ch_fus_w"], f32)
    sp_req_w = np.asarray(inputs["sp_req_w"], f32)
    sp_fus_w = np.asarray(inputs["sp_fus_w"], f32)
    cstv = np.zeros((NCONST,), f32)
    cstv[CI_SPW0:CI_SPW0 + 9] = sp_req_w[0, 0].reshape(-1) * f32(1.0 / 256.0)
    cstv[CI_SPW1:CI_SPW1 + 9] = sp_req_w[0, 1].reshape(-1)
    cstv[CI_WA] = sp_fus_w[0, 0]
    cstv[CI_NEGWA] = -sp_fus_w[0, 0]
    cstv[CI_WB] = sp_fus_w[0, 1]
    cstv[CI_B3] = np.asarray(inputs["st_b3"], f32)[0]
    cstv[CI_NEGB3] = -cstv[CI_B3]
    shared = {
        "w1t": np.ascontiguousarray(st_w1.T).astype(bf),
        "w2t": np.ascontiguousarray(st_w2.T).astype(bf),
        "w3t": np.ascontiguousarray(st_w3.T).astype(bf),
        "b1m": np.ascontiguousarray(np.asarray(inputs["st_b1"], f32).reshape(OB, 128).T),
        "b2m": np.ascontiguousarray(np.asarray(inputs["st_b2"], f32).reshape(OB, 128).T),
        "m1t": np.ascontiguousarray(np.asarray(inputs["mlp_w1"], f32).T),
        "m2t": np.ascontiguousarray(np.asarray(inputs["mlp_w2"], f32).T),
        "wat": np.ascontiguousarray(ch_fus_w[:, :C].T),
        "wbt": np.ascontiguousarray(ch_fus_w[:, C:].T),
        "cst": cstv,
    }
    in_maps = []
    for i in range(NCORES):
        vldv = np.zeros((RH, 1), f32)
        for r in range(RH):
            g = 16 * i + r - 2
            vldv[r, 0] = 1.0 if 0 <= g < H else 0.0
        m = dict(shared)
        m["feat"] = np.ascontiguousarray(padded[:, :, 16 * i:16 * i + RH, :])
        m["vld"] = vldv
        in_maps.append(m)
    return in_maps


def kernel(**inputs):
    if "nc" not in _CACHED:
        _CACHED["nc"] = build()
    nc = _CACHED["nc"]
    in_maps = _prep_inputs(inputs)
    res = run_bass_kernel_spmd(nc, in_maps, core_ids=list(range(NCORES)),
                               **_CACHED.get("run_kwargs", {}))
    _CACHED["last_result"] = res
    sparse_feature = np.empty((N, C, H, W), np.float32)
    sparse_mask = np.empty((N, C, H, W), np.float32)
    jsum = msum = mksum = 0.0
    for i in range(NCORES):
        r = res.results[i]
        sparse_feature[:, :, 16 * i:16 * (i + 1), :] = r["sf"]
        sparse_mask[:, :, 16 * i:16 * (i + 1), :] = r["sm"]
        jsum += float(r["stats"][0, 0])
        msum += float(r["stats"][0, 1])
        mksum += float(r["stats"][0, 2])
    npix = float(N * H * W)
    total_loss = np.float32(jsum / npix + msum / npix)
    mean_rate = np.float32(mksum / float((N - 1) * C * H * W))
    return (sparse_feature, total_loss, mean_rate, sparse_mask)


# revision 19
# speedup vs baseline: 1.3783x; 1.3783x over previous
"""Trainium2 Bass kernel for nn_Communication (gnn_message_passing).

Sharding: H=128 rows split 16/core across 8 NeuronCores; every core handles all
N=5 batch elements and all C=256 channels for its row slab. Each core receives
a 20-row slice (2-row halos, zero rows at the global image edges) so the SPMD
program is identical on every core. The only cross-core communication is one
AllGather of the per-core [128,20] channel-pool partials (avg-sums and maxes);
the scalar loss/mean-rate reductions return per-core partials that the host
sums.

Precision: attention/mask path in fp32; the statistics network (two 3-layer
pointwise MLPs, ~87% of FLOPs) runs bf16 on the TensorEngine with fp32 PSUM
accumulation. T and T' share feat/sparse bf16 tiles per (n, s-tile).
"""
import numpy as np
import ml_dtypes

import concourse.bass as bass
import concourse.tile as tile
import concourse.mybir as mybir
from concourse import bacc
from concourse.bass_utils import run_bass_kernel_spmd
from concourse.masks import make_identity

NCORES = 8
N, C, H, W = 5, 256, 128, 128
R = H // NCORES          # owned rows per core (16)
RH = R + 4               # rows incl 2-row halo (20)
KC = C // 128            # channel chunks (2)
OB = 8                   # 1024 hidden / 128
TS = 512                 # free-dim tile (4 rows x 128)
NT = (R * W) // TS       # s-tiles per core (4)
SFULL = RH * W           # 2560 (free size incl halo)
OWN0 = 2 * W             # first owned col (rows 2..18 of 20)
THRE = 0.01
F32 = mybir.dt.float32
BF16 = mybir.dt.bfloat16
FP8 = mybir.dt.float8e4
AX = mybir.AxisListType
OP = mybir.AluOpType
AF = mybir.ActivationFunctionType

# gaussian taps (compile-time constants, replicated from reference)
_c = 3 // 2
_x, _y = np.mgrid[-_c:3 - _c, -_c:3 - _c]
G2 = (1.0 / (2.0 * np.pi) * np.exp(-(_x ** 2 + _y ** 2) / 2.0)).astype(np.float32)
_g1x = np.arange(-1, 2, dtype=np.float32)
_g1 = np.exp(-_g1x ** 2 / 2.0)
G1 = (_g1 / _g1.sum()).astype(np.float32)

# consts vector indices
CI_SPW0 = 0     # 9 taps, in-ch 0 (pre-scaled by 1/256)
CI_SPW1 = 9     # 9 taps, in-ch 1
CI_WA = 18
CI_NEGWA = 19
CI_WB = 20
CI_B3 = 21
CI_NEGB3 = 22
NCONST = 24

_CACHED = {}


def _bf(ap2d, nrep):
    """Free-dim broadcast: [P,1] AP -> [P,nrep] via 0-stride."""
    return bass.AP(ap2d.tensor, ap2d.offset, [list(ap2d.ap[0]), [0, nrep]])


ACT_TABLE_PATCH = True


def _patch_act_tables():
    """Order activation tables so one table (Relu+Exp+Ln+Copy) serves the
    whole main loop -- avoids per-tile ACT table reloads."""
    import concourse.bacc as _bm
    if not ACT_TABLE_PATCH or getattr(_bm, "_act_tbl_patched", False):
        return
    _orig = _bm.get_activation_tables

    def _reordered(arch):
        t = _orig(arch)
        pref = 'natural_log_exp_and_others'
        order = [pref] + [k for k in t if k != pref]
        return {k: t[k] for k in order}

    _bm.get_activation_tables = _reordered
    _bm._act_tbl_patched = True


def build():
    _patch_act_tables()
    nc = bacc.Bacc("TRN2", target_bir_lowering=False, debug=False,
                   num_devices=NCORES)
    feat = nc.dram_tensor("feat", [N, C, RH, W], F32, kind="ExternalInput").ap()
    w1t = nc.dram_tensor("w1t", [2 * C, 1024], BF16, kind="ExternalInput").ap()
    w2t = nc.dram_tensor("w2t", [128, 4, 2, 1024], FP8, kind="ExternalInput").ap()
    w3t = nc.dram_tensor("w3t", [1024, 1], BF16, kind="ExternalInput").ap()
    b1m = nc.dram_tensor("b1m", [128, OB], F32, kind="ExternalInput").ap()
    b2m = nc.dram_tensor("b2m", [128, OB], F32, kind="ExternalInput").ap()
    m1t = nc.dram_tensor("m1t", [C, 16], F32, kind="ExternalInput").ap()
    m2t = nc.dram_tensor("m2t", [16, C], F32, kind="ExternalInput").ap()
    wat = nc.dram_tensor("wat", [C, C], F32, kind="ExternalInput").ap()
    wbt = nc.dram_tensor("wbt", [C, C], F32, kind="ExternalInput").ap()
    cst = nc.dram_tensor("cst", [NCONST], F32, kind="ExternalInput").ap()
    vld = nc.dram_tensor("vld", [RH, 1], F32, kind="ExternalInput").ap()

    sf = nc.dram_tensor("sf", [N, C, R, W], F32, kind="ExternalOutput").ap()
    sm = nc.dram_tensor("sm", [N, C, R, W], F32, kind="ExternalOutput").ap()
    stats = nc.dram_tensor("stats", [1, 4], F32, kind="ExternalOutput").ap()

    with tile.TileContext(nc) as tc:
        _build_body(nc, tc, feat, w1t, w2t, w3t, b1m, b2m, m1t, m2t, wat, wbt,
                    cst, vld, sf, sm, stats)
    nc.compile()
    return nc




def _build_body(nc, tc, feat, w1t, w2t, w3t, b1m, b2m, m1t, m2t, wat, wbt,
                cst, vld, sf, sm, stats):
    f32, bf16 = F32, BF16
    respool = tc.tile_pool(name="res", bufs=1)
    res = respool.__enter__()

    def _tc_tile(tc, shape, dtype, name):
        return res.tile(shape, dtype, tag=name, name=name)

    # ---------------- persistent tiles ----------------
    ft = _tc_tile(tc, [128, 2 * N, SFULL], f32, name="ft")          # resident feat
    w1s = _tc_tile(tc, [128, 4, 1024], bf16, name="w1s")
    w2s = _tc_tile(tc, [128, 4, 2, 1024], FP8, name="w2s")
    w3s = _tc_tile(tc, [128, OB, 1], bf16, name="w3s")
    b1s = _tc_tile(tc, [128, OB], f32, name="b1s")
    b2s = _tc_tile(tc, [128, OB], f32, name="b2s")
    m1s = _tc_tile(tc, [128, KC, 16], f32, name="m1s")
    m2s = _tc_tile(tc, [16, C], f32, name="m2s")
    was = _tc_tile(tc, [128, KC, C], f32, name="was")
    wbs = _tc_tile(tc, [128, KC, C], f32, name="wbs")
    cs = _tc_tile(tc, [128, NCONST], f32, name="cs")
    vst = _tc_tile(tc, [120, 1], f32, name="vst")
    ones = _tc_tile(tc, [128, TS], f32, name="ones")
    id4 = _tc_tile(tc, [4, 4], f32, name="id4")

    # conv buffers: 5 row-blocks of 24 partitions; data rows r=0..19 at
    # partition 24n+r, cols 1..128 hold W, col 0/129 stay zero.
    CW = 132
    cmsum = _tc_tile(tc, [120, CW], f32, name="cmsum")
    cmmax = _tc_tile(tc, [120, CW], f32, name="cmmax")
    sigm = _tc_tile(tc, [120, CW], f32, name="sigm")
    spatt = _tc_tile(tc, [120, CW], f32, name="spatt")
    ebuf = _tc_tile(tc, [24, CW], f32, name="ebuf")
    spcin = _tc_tile(tc, [120, CW], f32, name="spcin")
    spcoef = _tc_tile(tc, [120, CW], f32, name="spcoef")
    actm = _tc_tile(tc, [120, CW], f32, name="actm")
    spact = _tc_tile(tc, [120, CW], f32, name="spact")

    pool_p = _tc_tile(tc, [128, 20], f32, name="pool_p")            # local partials
    gsum = _tc_tile(tc, [128, 10], f32, name="gsum")
    gmax = _tc_tile(tc, [128, 10], f32, name="gmax")
    avs = _tc_tile(tc, [128, 10], f32, name="avs")
    chatt = _tc_tile(tc, [128, 10], f32, name="chatt")
    ego4 = _tc_tile(tc, [128, KC], f32, name="ego4")
    ego4m = _tc_tile(tc, [128, 4], f32, name="ego4m")
    chpad = _tc_tile(tc, [4, C + 2], f32, name="chpad")
    chc4 = _tc_tile(tc, [4, C], f32, name="chc4")
    chcoef = _tc_tile(tc, [128, 2 * 4], f32, name="chcoef")         # col kc*4+m
    jacc = _tc_tile(tc, [1, N * NT], f32, name="jacc")
    macc = _tc_tile(tc, [1, N * NT], f32, name="macc")
    mks = _tc_tile(tc, [128, 4 * KC * NT], f32, name="mks")
    statsb = _tc_tile(tc, [1, 4], f32, name="statsb")
    spscr = _tc_tile(tc, [1, TS], f32, name="spscr")
    spscr2 = _tc_tile(tc, [1, TS], f32, name="spscr2")

    for t_ in (cmsum, cmmax, sigm, spatt, spcin, spcoef, actm, spact, ebuf,
               vst, statsb):
        nc.vector.memset(t_, 0.0)
    nc.vector.memset(ones, 1.0)
    make_identity(nc, id4)

    # ---------------- input DMAs ----------------
    for n in range(N):
        for kc in range(KC):
            nc.sync.dma_start(
                out=ft[:, n * KC + kc, :],
                in_=feat[n, kc * 128:(kc + 1) * 128, :, :])
    nc.sync.dma_start(out=w1s, in_=bass.AP(
        w1t.tensor, 0, [[1024, 128], [128 * 1024, 4], [1, 1024]]))
    nc.sync.dma_start(out=w2s, in_=w2t)
    nc.sync.dma_start(out=w3s, in_=bass.AP(
        w3t.tensor, 0, [[1, 128], [128, OB], [1, 1]]))
    nc.sync.dma_start(out=b1s, in_=b1m)
    nc.sync.dma_start(out=b2s, in_=b2m)
    nc.sync.dma_start(out=m1s, in_=bass.AP(
        m1t.tensor, 0, [[16, 128], [128 * 16, KC], [1, 16]]))
    nc.sync.dma_start(out=m2s, in_=m2t)
    nc.sync.dma_start(out=was, in_=bass.AP(
        wat.tensor, 0, [[C, 128], [128 * C, KC], [1, C]]))
    nc.sync.dma_start(out=wbs, in_=bass.AP(
        wbt.tensor, 0, [[C, 128], [128 * C, KC], [1, C]]))
    nc.sync.dma_start(out=cs, in_=bass.AP(
        cst.tensor, 0, [[0, 128], [1, NCONST]]))
    for n in range(N):
        nc.sync.dma_start(out=vst[24 * n:24 * n + RH, 0:1], in_=vld)

    def c1(j, lo, hi):
        return cs[lo:hi, j:j + 1]

    # ---------------- phase 0: pools + collective ----------------
    for n in range(N):
        for kc in range(KC):
            i = n * KC + kc
            nc.vector.reduce_sum(out=pool_p[:, kc * N + n: kc * N + n + 1],
                                 in_=ft[:, i, OWN0:OWN0 + R * W], axis=AX.X)
            nc.vector.tensor_reduce(out=pool_p[:, 10 + kc * N + n: 11 + kc * N + n],
                                    in_=ft[:, i, OWN0:OWN0 + R * W], axis=AX.X,
                                    op=OP.max)
    with tc.tile_pool(name="ccd", bufs=1, space="DRAM") as ccd:
        cc_in = ccd.tile([128, 20], f32)
        cc_out = ccd.tile([NCORES * 128, 20], f32, addr_space="Shared")
        nc.sync.dma_start(out=cc_in, in_=pool_p)
        nc.gpsimd.collective_compute(
            "AllGather", OP.bypass,
            replica_groups=[list(range(NCORES))],
            ins=[cc_in.opt()], outs=[cc_out.opt()])
        gath = _tc_tile(tc, [128, NCORES, 20], f32, name="gath")
        nc.sync.dma_start(out=gath, in_=bass.AP(
            cc_out.tensor, cc_out.offset, [[20, 128], [128 * 20, NCORES], [1, 20]]))
    # reduce over the 8 gathered shards
    nc.vector.reduce_sum(out=gsum, in_=bass.AP(
        gath.tensor, gath.offset, [list(gath.ap[0]), [1, 10], [20, NCORES]]),
        axis=AX.X)
    nc.vector.tensor_reduce(out=gmax, in_=bass.AP(
        gath.tensor, gath.offset + 10, [list(gath.ap[0]), [1, 10], [20, NCORES]]),
        axis=AX.X, op=OP.max)

    # ---------------- phase 0b: channel maps + convs (fp32, DVE) ----------
    from concourse import bass_isa
    with tc.tile_pool(name="tree", bufs=1) as tree, \
         tc.tile_pool(name="csps", bufs=2, space="PSUM") as csps:
        for n in range(N):
            # channel sums on the (otherwise idle) TensorEngine
            csrow = tree.tile([1, SFULL], f32, tag="csrow")
            for ch in range(SFULL // TS):
                cps = csps.tile([1, TS], f32, tag="csum")
                for kc in range(KC):
                    nc.tensor.matmul(cps, ones[:, 0:1],
                                     ft[:, n * KC + kc, ch * TS:(ch + 1) * TS],
                                     start=(kc == 0), stop=(kc == 1))
                nc.scalar.activation(out=csrow[0:1, ch * TS:(ch + 1) * TS],
                                     in_=cps, func=AF.Copy)
            nc.sync.dma_start(
                out=cmsum[24 * n:24 * n + RH, 1:129],
                in_=csrow[0:1, 0:SFULL])
            # channel maxes via gpsimd partition all-reduce
            tmp = tree.tile([128, SFULL], f32, tag="tmp")
            nc.vector.tensor_tensor(out=tmp, in0=ft[:, n * KC, :],
                                    in1=ft[:, n * KC + 1, :], op=OP.max)
            nc.gpsimd.partition_all_reduce(out_ap=tmp[:], in_ap=tmp[:],
                                           channels=128,
                                           reduce_op=bass_isa.ReduceOp.max)
            nc.sync.dma_start(
                out=cmmax[24 * n:24 * n + RH, 1:129],
                in_=tmp[0:1, 0:SFULL])

    cscr = _tc_tile(tc, [120, CW], f32, name="cscr")
    nc.vector.memset(cscr, 0.0)

    def conv3(dst, srcs_taps):
        """dst[0:120,1:129] = sum over (src, taps9): taps[3dy+dx]*src[p+dy-1, dx:dx+128].
        Row (partition) shifts go through a DMA copy; every compute op is
        base-partition-0 over all 120 rows (junk rows are never consumed)."""
        first = True
        for src, taps in srcs_taps:
            for dy in range(3):
                if dy == 1:
                    sb = src
                else:
                    if dy == 0:
                        nc.sync.dma_start(out=cscr[1:120, :], in_=src[0:119, :])
                    else:
                        nc.sync.dma_start(out=cscr[0:119, :], in_=src[1:120, :])
                    sb = cscr
                for dx in range(3):
                    tap = taps[3 * dy + dx]
                    in0 = sb[0:120, dx:dx + 128]
                    o = dst[0:120, 1:129]
                    if isinstance(tap, (float, int)):
                        s = float(tap)
                    else:
                        s = c1(tap, 0, 120)
                    if first:
                        nc.vector.tensor_scalar(out=o, in0=in0, scalar1=s,
                                                scalar2=None, op0=OP.mult)
                        first = False
                    else:
                        nc.vector.scalar_tensor_tensor(
                            out=o, in0=in0, scalar=s, in1=o,
                            op0=OP.mult, op1=OP.add)

    # sp_att = sigmoid(conv([mean, max]))
    conv3(spatt, [(cmsum, [CI_SPW0 + j for j in range(9)]),
                  (cmmax, [CI_SPW1 + j for j in range(9)])])
    nc.scalar.activation(out=spatt[0:120, 1:129], in_=spatt[0:120, 1:129],
                         func=AF.Sigmoid)
    # act = conv(sigmoid(mean/256) * valid, g2)
    nc.scalar.activation(out=sigm[0:120, 1:129], in_=cmsum[0:120, 1:129],
                         func=AF.Sigmoid, scale=1.0 / 256.0)
    nc.vector.tensor_scalar(out=sigm[0:120, 1:129], in0=sigm[0:120, 1:129],
                            scalar1=vst[0:120, 0:1], scalar2=None, op0=OP.mult)
    conv3(actm, [(sigm, [float(G2[dy, dx]) for dy in range(3) for dx in range(3)])])
    # sp_coef = conv(sigmoid(wa*ego + wb*sp_att[1:]) * valid, g2)
    nc.vector.tensor_scalar(out=ebuf[0:20, 1:129], in0=spatt[0:20, 1:129],
                            scalar1=c1(CI_NEGWA, 0, 20), scalar2=c1(CI_WA, 0, 20),
                            op0=OP.mult, op1=OP.add)
    for m in range(1, N):
        nc.sync.dma_start(out=spcin[24 * m:24 * m + 20, 1:129],
                          in_=ebuf[0:20, 1:129])
    nc.vector.scalar_tensor_tensor(
        out=spcin[0:120, 1:129], in0=spatt[0:120, 1:129],
        scalar=c1(CI_WB, 0, 120), in1=spcin[0:120, 1:129],
        op0=OP.mult, op1=OP.add)
    nc.scalar.activation(out=spcin[0:120, 1:129], in_=spcin[0:120, 1:129],
                         func=AF.Sigmoid)
    nc.vector.tensor_scalar(out=spcin[0:120, 1:129], in0=spcin[0:120, 1:129],
                            scalar1=vst[0:120, 0:1], scalar2=None, op0=OP.mult)
    conv3(spcoef, [(spcin, [float(G2[dy, dx]) for dy in range(3) for dx in range(3)])])
    nc.vector.tensor_tensor(out=spact[0:120, 1:129], in0=spcoef[0:120, 1:129],
                            in1=actm[0:120, 1:129], op=OP.mult)
    # flatten spact rows into free dim (single partition) for later broadcast
    spact_bf = _tc_tile(tc, [120, CW], bf16, name="spact_bf")
    nc.vector.tensor_copy(out=spact_bf[0:120, :], in_=spact[0:120, :])
    _saflcm = tc.tile_pool(name="safld", bufs=1, space="DRAM")
    saflpool = _saflcm.__enter__()
    safl = saflpool.tile([4, R * W], bf16)
    for m in range(4):
        nc.sync.dma_start(out=safl[m:m + 1, :],
                          in_=spact_bf[24 * (m + 1) + 2:24 * (m + 1) + 18, 1:129])

    # ---------------- phase 0c: MLP -> ch_att -> ch_coef ----------------
    with tc.tile_pool(name="p0ps", bufs=2, space="PSUM") as p0ps, \
         tc.tile_pool(name="p0sb", bufs=2) as p0sb:
        nc.scalar.mul(out=avs, in_=gsum, mul=1.0 / float(H * W))
        hidps = p0ps.tile([16, N], f32, tag="hid")
        for kc in range(KC):
            nc.tensor.matmul(hidps, m1s[:, kc, :], avs[:, kc * N:(kc + 1) * N],
                             start=(kc == 0), stop=(kc == 1))
        hida = p0sb.tile([16, N], f32, tag="hid_sb")
        nc.scalar.activation(out=hida, in_=hidps, func=AF.Relu)
        hidps2 = p0ps.tile([16, N], f32, tag="hid")
        for kc in range(KC):
            nc.tensor.matmul(hidps2, m1s[:, kc, :], gmax[:, kc * N:(kc + 1) * N],
                             start=(kc == 0), stop=(kc == 1))
        hidm = p0sb.tile([16, N], f32, tag="hid_sb")
        nc.scalar.activation(out=hidm, in_=hidps2, func=AF.Relu)
        for kco in range(KC):
            chps = p0ps.tile([128, N], f32, tag="chps")
            nc.tensor.matmul(chps, m2s[:, kco * 128:(kco + 1) * 128], hida,
                             start=True, stop=False)
            nc.tensor.matmul(chps, m2s[:, kco * 128:(kco + 1) * 128], hidm,
                             start=False, stop=True)
            nc.scalar.activation(out=chatt[:, kco * N:(kco + 1) * N], in_=chps,
                                 func=AF.Sigmoid)
        for kc in range(KC):
            nc.vector.tensor_scalar(out=ego4[:, kc:kc + 1],
                                    in0=chatt[:, kc * N:kc * N + 1],
                                    scalar1=-1.0, scalar2=1.0,
                                    op0=OP.mult, op1=OP.add)
        # ch_coef_pre[m, c] (m=0..3 -> n=1..4)
        chpre = p0ps.tile([4, C], f32, tag="chpre")
        for kc in range(KC):
            nc.vector.tensor_copy(out=ego4m[:, :], in_=_bf(ego4[:, kc:kc + 1], 4))
            nc.tensor.matmul(chpre, ego4m, was[:, kc, :],
                             start=(kc == 0), stop=False)
            nc.tensor.matmul(chpre, chatt[:, kc * N + 1:kc * N + N],
                             wbs[:, kc, :], start=False, stop=(kc == 1))
        nc.vector.memset(chpad, 0.0)
        nc.scalar.activation(out=chpad[:, 1:C + 1], in_=chpre, func=AF.Sigmoid)
        # 3-tap gaussian smoothing along C
        nc.vector.tensor_scalar(out=chc4, in0=chpad[:, 1:C + 1],
                                scalar1=float(G1[1]), scalar2=None, op0=OP.mult)
        nc.vector.scalar_tensor_tensor(out=chc4, in0=chpad[:, 0:C],
                                       scalar=float(G1[0]), in1=chc4,
                                       op0=OP.mult, op1=OP.add)
        nc.vector.scalar_tensor_tensor(out=chc4, in0=chpad[:, 2:C + 2],
                                       scalar=float(G1[2]), in1=chc4,
                                       op0=OP.mult, op1=OP.add)
        for kc in range(KC):
            tp = p0ps.tile([128, 4], f32, tag="tp")
            nc.tensor.transpose(tp, chc4[:, kc * 128:(kc + 1) * 128], id4)
            nc.scalar.activation(out=chcoef[:, kc * 4:(kc + 1) * 4], in_=tp,
                                 func=AF.Copy)

    # ---------------- main loop ----------------
    with tc.tile_pool(name="fbf", bufs=14) as fbfp, \
         tc.tile_pool(name="spb", bufs=3) as spbp, \
         tc.tile_pool(name="mstage", bufs=2) as mst, \
         tc.tile_pool(name="h1p", bufs=2) as h1p, \
         tc.tile_pool(name="h2p", bufs=2) as h2p, \
         tc.tile_pool(name="bcp", bufs=2) as bcp, \
         tc.tile_pool(name="psA", bufs=3, space="PSUM") as psA, \
         tc.tile_pool(name="psB", bufs=2, space="PSUM") as psB, \
         tc.tile_pool(name="psT", bufs=3, space="PSUM") as psT:

        fbf = {}

        def cast_feat(n, eng):
            for kc in range(KC):
                for t in range(NT):
                    tl = fbfp.tile([128, TS], bf16, tag="fbf")
                    eng.tensor_copy(
                        out=tl, in_=ft[:, n * KC + kc, OWN0 + t * TS:OWN0 + (t + 1) * TS])
                    fbf[(n, kc, t)] = tl

        cast_feat(0, nc.vector)
        cast_feat(1, nc.vector)

        for n in range(N):
            nxt = (n + 1) % N
            if 1 <= n <= N - 2:
                cast_feat(n + 1, nc.vector)
            elif n == N - 1:
                cast_feat(0, nc.vector)  # gen-0 tiles were recycled
            for t in range(NT):
                # ---- mask / sparse for this (n, t) ----
                if n == 0:
                    spbf = [fbf[(0, kc, t)] for kc in range(KC)]
                    for kc in range(KC):
                        nc.sync.dma_start(
                            out=sf[0, kc * 128:(kc + 1) * 128, 4 * t:4 * t + 4, :],
                            in_=ft[:, kc, OWN0 + t * TS:OWN0 + (t + 1) * TS])
                        nc.sync.dma_start(
                            out=sm[0, kc * 128:(kc + 1) * 128, 4 * t:4 * t + 4, :],
                            in_=ones)
                else:
                    m = n - 1
                    sab = bcp.tile([128, TS], bf16, tag="sab")
                    nc.sync.dma_start(out=sab, in_=bass.AP(
                        safl.tensor, safl.offset + m * (R * W) + t * TS,
                        [[0, 128], [1, TS]]))
                    spbf = []
                    for kc in range(KC):
                        mtile = mst.tile([128, TS], f32, tag="msk")
                        nc.vector.tensor_scalar(
                            out=mtile, in0=sab,
                            scalar1=chcoef[:, kc * 4 + m:kc * 4 + m + 1],
                            scalar2=THRE, op0=OP.mult, op1=OP.is_gt)
                        nc.vector.reduce_sum(
                            out=mks[:, m * KC * NT + kc * NT + t:
                                    m * KC * NT + kc * NT + t + 1],
                            in_=mtile, axis=AX.X)
                        nc.sync.dma_start(
                            out=sm[n, kc * 128:(kc + 1) * 128, 4 * t:4 * t + 4, :],
                            in_=mtile)
                        spf = mst.tile([128, TS], f32, tag="spf")
                        nc.vector.tensor_tensor(
                            out=spf, in0=mtile,
                            in1=ft[:, n * KC + kc, OWN0 + t * TS:OWN0 + (t + 1) * TS],
                            op=OP.mult)
                        nc.sync.dma_start(
                            out=sf[n, kc * 128:(kc + 1) * 128, 4 * t:4 * t + 4, :],
                            in_=spf)
                        sbf = spbp.tile([128, TS], bf16, tag="spb")
                        nc.vector.tensor_copy(out=sbf, in_=spf)
                        spbf.append(sbf)

                # ---- layer 1 (pair L=T[n], R=T'[n]) ----
                h1L = h1p.tile([128, OB, TS], FP8, tag="h1L")
                h1R = h1p.tile([128, OB, TS], FP8, tag="h1R")
                for ob in range(OB):
                    oc = slice(ob * 128, (ob + 1) * 128)
                    zL = psA.tile([128, TS], f32, tag="za")
                    zR = psA.tile([128, TS], f32, tag="za")
                    rhsL = [fbf[(n, 0, t)], fbf[(n, 1, t)], spbf[0], spbf[1]]
                    rhsR = [fbf[(nxt, 0, t)], fbf[(nxt, 1, t)], spbf[0], spbf[1]]
                    for r in range(4):
                        nc.tensor.matmul(zL, w1s[:, r, oc], rhsL[r],
                                         start=(r == 0), stop=(r == 3))
                        nc.tensor.matmul(zR, w1s[:, r, oc], rhsR[r],
                                         start=(r == 0), stop=(r == 3))
                    nc.scalar.activation(out=h1L[:, ob, :], in_=zL, func=AF.Relu,
                                         bias=b1s[:, ob:ob + 1])
                    nc.vector.tensor_scalar(out=h1R[:, ob, :], in0=zR,
                                            scalar1=b1s[:, ob:ob + 1],
                                            scalar2=0.0, op0=OP.add, op1=OP.max)

                # ---- layer 2 + layer 3 ----
                TL = psT.tile([1, TS], f32, tag="T")
                TR = psT.tile([1, TS], f32, tag="T")
                for ob in range(OB):
                    oc = slice(ob * 128, (ob + 1) * 128)
                    z2L = psB.tile([128, TS], f32, tag="zb")
                    z2R = psB.tile([128, TS], f32, tag="zb")
                    for j in range(4):
                        nc.tensor.matmul(z2L, w2s[:, j, :, oc],
                                         h1L[:, 2 * j:2 * j + 2, :],
                                         start=(j == 0), stop=(j == 3),
                                         perf_mode=mybir.MatmulPerfMode.DoubleRow)
                        nc.tensor.matmul(z2R, w2s[:, j, :, oc],
                                         h1R[:, 2 * j:2 * j + 2, :],
                                         start=(j == 0), stop=(j == 3),
                                         perf_mode=mybir.MatmulPerfMode.DoubleRow)
                    h2L = h2p.tile([128, TS], bf16, tag="h2L")
                    nc.scalar.activation(out=h2L, in_=z2L, func=AF.Relu,
                                         bias=b2s[:, ob:ob + 1])
                    h2R = h2p.tile([128, TS], bf16, tag="h2R")
                    nc.vector.tensor_scalar(out=h2R, in0=z2R,
                                            scalar1=b2s[:, ob:ob + 1],
                                            scalar2=0.0, op0=OP.add, op1=OP.max)
                    nc.tensor.matmul(TL, w3s[:, ob, :], h2L,
                                     start=(ob == 0), stop=(ob == 7))
                    nc.tensor.matmul(TR, w3s[:, ob, :], h2R,
                                     start=(ob == 0), stop=(ob == 7))
                # softplus partials
                idx = n * NT + t
                # softplus(x) = ln(1 + e^x); x = -(T+b3) for joint, +(T'+b3)
                nc.scalar.activation(out=spscr, in_=TL, func=AF.Exp,
                                     scale=-1.0, bias=c1(CI_NEGB3, 0, 1))
                nc.scalar.activation(out=spscr2, in_=TR, func=AF.Exp,
                                     scale=1.0, bias=c1(CI_B3, 0, 1))
                nc.scalar.activation(out=spscr, in_=spscr, func=AF.Ln,
                                     bias=1.0, accum_out=jacc[0:1, idx:idx + 1])
                nc.scalar.activation(out=spscr2, in_=spscr2, func=AF.Ln,
                                     bias=1.0, accum_out=macc[0:1, idx:idx + 1])

        # ---- final scalar partials ----
        nc.vector.reduce_sum(out=statsb[0:1, 0:1], in_=jacc, axis=AX.X)
        nc.vector.reduce_sum(out=statsb[0:1, 1:2], in_=macc, axis=AX.X)
        mkcol = _tc_tile(tc, [128, 1], f32, name="mkcol")
        nc.vector.reduce_sum(out=mkcol, in_=mks, axis=AX.X)
        stps = psT.tile([1, TS], f32, tag="T")
        nc.tensor.matmul(stps[0:1, 0:1], ones[:, 0:1], mkcol,
                         start=True, stop=True)
        nc.scalar.activation(out=statsb[0:1, 2:3], in_=stps[0:1, 0:1],
                             func=AF.Copy)
        nc.sync.dma_start(out=stats, in_=statsb)


def _prep_inputs(inputs):
    f32 = np.float32
    bf = ml_dtypes.bfloat16
    feat = np.ascontiguousarray(inputs["feat"], dtype=f32)
    padded = np.zeros((N, C, H + 4, W), f32)
    padded[:, :, 2:H + 2, :] = feat
    st_w1 = np.asarray(inputs["st_w1"], f32)
    st_w2 = np.asarray(inputs["st_w2"], f32)
    st_w3 = np.asarray(inputs["st_w3"], f32)
    ch_fus_w = np.asarray(inputs["ch_fus_w"], f32)
    sp_req_w = np.asarray(inputs["sp_req_w"], f32)
    sp_fus_w = np.asarray(inputs["sp_fus_w"], f32)
    cstv = np.zeros((NCONST,), f32)
    cstv[CI_SPW0:CI_SPW0 + 9] = sp_req_w[0, 0].reshape(-1) * f32(1.0 / 256.0)
    cstv[CI_SPW1:CI_SPW1 + 9] = sp_req_w[0, 1].reshape(-1)
    cstv[CI_WA] = sp_fus_w[0, 0]
    cstv[CI_NEGWA] = -sp_fus_w[0, 0]
    cstv[CI_WB] = sp_fus_w[0, 1]
    cstv[CI_B3] = np.asarray(inputs["st_b3"], f32)[0]
    cstv[CI_NEGB3] = -cstv[CI_B3]
    shared = {
        "w1t": np.ascontiguousarray(st_w1.T).astype(bf),
        "w2t": np.ascontiguousarray(
            st_w2.T.reshape(4, 2, 128, 1024).transpose(2, 0, 1, 3)
        ).astype(ml_dtypes.float8_e4m3),
        "w3t": np.ascontiguousarray(st_w3.T).astype(bf),
        "b1m": np.ascontiguousarray(np.asarray(inputs["st_b1"], f32).reshape(OB, 128).T),
        "b2m": np.ascontiguousarray(np.asarray(inputs["st_b2"], f32).reshape(OB, 128).T),
        "m1t": np.ascontiguousarray(np.asarray(inputs["mlp_w1"], f32).T),
        "m2t": np.ascontiguousarray(np.asarray(inputs["mlp_w2"], f32).T),
        "wat": np.ascontiguousarray(ch_fus_w[:, :C].T),
        "wbt": np.ascontiguousarray(ch_fus_w[:, C:].T),
        "cst": cstv,
    }
    in_maps = []
    for i in range(NCORES):
        vldv = np.zeros((RH, 1), f32)
        for r in range(RH):
            g = 16 * i + r - 2
            vldv[r, 0] = 1.0 if 0 <= g < H else 0.0
        m = dict(shared)
        m["feat"] = np.ascontiguousarray(padded[:, :, 16 * i:16 * i + RH, :])
        m["vld"] = vldv
        in_maps.append(m)
    return in_maps


def kernel(**inputs):
    if "nc" not in _CACHED:
        _CACHED["nc"] = build()
    nc = _CACHED["nc"]
    in_maps = _prep_inputs(inputs)
    res = run_bass_kernel_spmd(nc, in_maps, core_ids=list(range(NCORES)),
                               **_CACHED.get("run_kwargs", {}))
    _CACHED["last_result"] = res
    sparse_feature = np.empty((N, C, H, W), np.float32)
    sparse_mask = np.empty((N, C, H, W), np.float32)
    jsum = msum = mksum = 0.0
    for i in range(NCORES):
        r = res.results[i]
        sparse_feature[:, :, 16 * i:16 * (i + 1), :] = r["sf"]
        sparse_mask[:, :, 16 * i:16 * (i + 1), :] = r["sm"]
        jsum += float(r["stats"][0, 0])
        msum += float(r["stats"][0, 1])
        mksum += float(r["stats"][0, 2])
    npix = float(N * H * W)
    total_loss = np.float32(jsum / npix + msum / npix)
    mean_rate = np.float32(mksum / float((N - 1) * C * H * W))
    return (sparse_feature, total_loss, mean_rate, sparse_mask)


# revision 20
# speedup vs baseline: 1.6396x; 1.1895x over previous
"""Trainium2 Bass kernel for nn_Communication (gnn_message_passing).

Sharding: H=128 rows split 16/core across 8 NeuronCores; every core handles all
N=5 batch elements and all C=256 channels for its row slab. Each core receives
a 20-row slice (2-row halos, zero rows at the global image edges) so the SPMD
program is identical on every core. The only cross-core communication is one
AllGather of the per-core [128,20] channel-pool partials (avg-sums and maxes);
the scalar loss/mean-rate reductions return per-core partials that the host
sums.

Precision: attention/mask path in fp32; the statistics network (two 3-layer
pointwise MLPs, ~87% of FLOPs) runs bf16 on the TensorEngine with fp32 PSUM
accumulation. T and T' share feat/sparse bf16 tiles per (n, s-tile).
"""
import numpy as np
import ml_dtypes

import concourse.bass as bass
import concourse.tile as tile
import concourse.mybir as mybir
from concourse import bacc
from concourse.bass_utils import run_bass_kernel_spmd
from concourse.masks import make_identity

NCORES = 8
N, C, H, W = 5, 256, 128, 128
R = H // NCORES          # owned rows per core (16)
RH = R + 4               # rows incl 2-row halo (20)
KC = C // 128            # channel chunks (2)
OB = 8                   # 1024 hidden / 128
TS = 512                 # free-dim tile (4 rows x 128)
NT = (R * W) // TS       # s-tiles per core (4)
SFULL = RH * W           # 2560 (free size incl halo)
OWN0 = 2 * W             # first owned col (rows 2..18 of 20)
THRE = 0.01
F32 = mybir.dt.float32
BF16 = mybir.dt.bfloat16
FP8 = mybir.dt.float8e4
AX = mybir.AxisListType
OP = mybir.AluOpType
AF = mybir.ActivationFunctionType

# gaussian taps (compile-time constants, replicated from reference)
_c = 3 // 2
_x, _y = np.mgrid[-_c:3 - _c, -_c:3 - _c]
G2 = (1.0 / (2.0 * np.pi) * np.exp(-(_x ** 2 + _y ** 2) / 2.0)).astype(np.float32)
_g1x = np.arange(-1, 2, dtype=np.float32)
_g1 = np.exp(-_g1x ** 2 / 2.0)
G1 = (_g1 / _g1.sum()).astype(np.float32)

# consts vector indices
CI_SPW0 = 0     # 9 taps, in-ch 0 (pre-scaled by 1/256)
CI_SPW1 = 9     # 9 taps, in-ch 1
CI_WA = 18
CI_NEGWA = 19
CI_WB = 20
CI_B3 = 21
CI_NEGB3 = 22
NCONST = 24

_CACHED = {}


def _bf(ap2d, nrep):
    """Free-dim broadcast: [P,1] AP -> [P,nrep] via 0-stride."""
    return bass.AP(ap2d.tensor, ap2d.offset, [list(ap2d.ap[0]), [0, nrep]])


ACT_TABLE_PATCH = True


def _patch_act_tables():
    """Order activation tables so one table (Relu+Exp+Ln+Copy) serves the
    whole main loop -- avoids per-tile ACT table reloads."""
    import concourse.bacc as _bm
    if not ACT_TABLE_PATCH or getattr(_bm, "_act_tbl_patched", False):
        return
    _orig = _bm.get_activation_tables

    def _reordered(arch):
        t = _orig(arch)
        pref = 'natural_log_exp_and_others'
        order = [pref] + [k for k in t if k != pref]
        return {k: t[k] for k in order}

    _bm.get_activation_tables = _reordered
    _bm._act_tbl_patched = True


def build():
    _patch_act_tables()
    nc = bacc.Bacc("TRN2", target_bir_lowering=False, debug=False,
                   num_devices=NCORES)
    feat = nc.dram_tensor("feat", [N, C, RH, W], F32, kind="ExternalInput").ap()
    w1t = nc.dram_tensor("w1t", [128, 2, 2, 1024], FP8, kind="ExternalInput").ap()
    w2t = nc.dram_tensor("w2t", [128, 4, 2, 1024], FP8, kind="ExternalInput").ap()
    w3t = nc.dram_tensor("w3t", [1024, 1], BF16, kind="ExternalInput").ap()
    b1m = nc.dram_tensor("b1m", [128, OB], F32, kind="ExternalInput").ap()
    b2m = nc.dram_tensor("b2m", [128, OB], F32, kind="ExternalInput").ap()
    m1t = nc.dram_tensor("m1t", [C, 16], F32, kind="ExternalInput").ap()
    m2t = nc.dram_tensor("m2t", [16, C], F32, kind="ExternalInput").ap()
    wat = nc.dram_tensor("wat", [C, C], F32, kind="ExternalInput").ap()
    wbt = nc.dram_tensor("wbt", [C, C], F32, kind="ExternalInput").ap()
    cst = nc.dram_tensor("cst", [NCONST], F32, kind="ExternalInput").ap()
    vld = nc.dram_tensor("vld", [RH, 1], F32, kind="ExternalInput").ap()

    sf = nc.dram_tensor("sf", [N, C, R, W], F32, kind="ExternalOutput").ap()
    sm = nc.dram_tensor("sm", [N, C, R, W], F32, kind="ExternalOutput").ap()
    stats = nc.dram_tensor("stats", [1, 4], F32, kind="ExternalOutput").ap()

    with tile.TileContext(nc) as tc:
        _build_body(nc, tc, feat, w1t, w2t, w3t, b1m, b2m, m1t, m2t, wat, wbt,
                    cst, vld, sf, sm, stats)
    nc.compile()
    return nc




def _build_body(nc, tc, feat, w1t, w2t, w3t, b1m, b2m, m1t, m2t, wat, wbt,
                cst, vld, sf, sm, stats):
    f32, bf16 = F32, BF16
    respool = tc.tile_pool(name="res", bufs=1)
    res = respool.__enter__()

    def _tc_tile(tc, shape, dtype, name):
        return res.tile(shape, dtype, tag=name, name=name)

    # ---------------- persistent tiles ----------------
    ft = _tc_tile(tc, [128, 2 * N, SFULL], f32, name="ft")          # resident feat
    w1s = _tc_tile(tc, [128, 2, 2, 1024], FP8, name="w1s")
    w2s = _tc_tile(tc, [128, 4, 2, 1024], FP8, name="w2s")
    w3s = _tc_tile(tc, [128, OB, 1], bf16, name="w3s")
    b1s = _tc_tile(tc, [128, OB], f32, name="b1s")
    b2s = _tc_tile(tc, [128, OB], f32, name="b2s")
    m1s = _tc_tile(tc, [128, KC, 16], f32, name="m1s")
    m2s = _tc_tile(tc, [16, C], f32, name="m2s")
    was = _tc_tile(tc, [128, KC, C], f32, name="was")
    wbs = _tc_tile(tc, [128, KC, C], f32, name="wbs")
    cs = _tc_tile(tc, [128, NCONST], f32, name="cs")
    vst = _tc_tile(tc, [120, 1], f32, name="vst")
    ones = _tc_tile(tc, [128, TS], f32, name="ones")
    id4 = _tc_tile(tc, [4, 4], f32, name="id4")

    # conv buffers: 5 row-blocks of 24 partitions; data rows r=0..19 at
    # partition 24n+r, cols 1..128 hold W, col 0/129 stay zero.
    CW = 132
    cmsum = _tc_tile(tc, [120, CW], f32, name="cmsum")
    cmmax = _tc_tile(tc, [120, CW], f32, name="cmmax")
    sigm = _tc_tile(tc, [120, CW], f32, name="sigm")
    spatt = _tc_tile(tc, [120, CW], f32, name="spatt")
    ebuf = _tc_tile(tc, [24, CW], f32, name="ebuf")
    spcin = _tc_tile(tc, [120, CW], f32, name="spcin")
    spcoef = _tc_tile(tc, [120, CW], f32, name="spcoef")
    actm = _tc_tile(tc, [120, CW], f32, name="actm")
    spact = _tc_tile(tc, [120, CW], f32, name="spact")

    pool_p = _tc_tile(tc, [128, 20], f32, name="pool_p")            # local partials
    gsum = _tc_tile(tc, [128, 10], f32, name="gsum")
    gmax = _tc_tile(tc, [128, 10], f32, name="gmax")
    avs = _tc_tile(tc, [128, 10], f32, name="avs")
    chatt = _tc_tile(tc, [128, 10], f32, name="chatt")
    ego4 = _tc_tile(tc, [128, KC], f32, name="ego4")
    ego4m = _tc_tile(tc, [128, 4], f32, name="ego4m")
    chpad = _tc_tile(tc, [4, C + 2], f32, name="chpad")
    chc4 = _tc_tile(tc, [4, C], f32, name="chc4")
    chcoef = _tc_tile(tc, [128, 2 * 4], f32, name="chcoef")         # col kc*4+m
    jacc = _tc_tile(tc, [1, N * NT], f32, name="jacc")
    macc = _tc_tile(tc, [1, N * NT], f32, name="macc")
    mks = _tc_tile(tc, [128, 4 * KC * NT], f32, name="mks")
    statsb = _tc_tile(tc, [1, 4], f32, name="statsb")
    spscr = _tc_tile(tc, [1, TS], f32, name="spscr")
    spscr2 = _tc_tile(tc, [1, TS], f32, name="spscr2")

    for t_ in (cmsum, cmmax, sigm, spatt, spcin, spcoef, actm, spact, ebuf,
               vst, statsb):
        nc.vector.memset(t_, 0.0)
    nc.vector.memset(ones, 1.0)
    make_identity(nc, id4)

    # ---------------- input DMAs ----------------
    for n in range(N):
        for kc in range(KC):
            nc.sync.dma_start(
                out=ft[:, n * KC + kc, :],
                in_=feat[n, kc * 128:(kc + 1) * 128, :, :])
    nc.sync.dma_start(out=w1s, in_=w1t)
    nc.sync.dma_start(out=w2s, in_=w2t)
    nc.sync.dma_start(out=w3s, in_=bass.AP(
        w3t.tensor, 0, [[1, 128], [128, OB], [1, 1]]))
    nc.sync.dma_start(out=b1s, in_=b1m)
    nc.sync.dma_start(out=b2s, in_=b2m)
    nc.sync.dma_start(out=m1s, in_=bass.AP(
        m1t.tensor, 0, [[16, 128], [128 * 16, KC], [1, 16]]))
    nc.sync.dma_start(out=m2s, in_=m2t)
    nc.sync.dma_start(out=was, in_=bass.AP(
        wat.tensor, 0, [[C, 128], [128 * C, KC], [1, C]]))
    nc.sync.dma_start(out=wbs, in_=bass.AP(
        wbt.tensor, 0, [[C, 128], [128 * C, KC], [1, C]]))
    nc.sync.dma_start(out=cs, in_=bass.AP(
        cst.tensor, 0, [[0, 128], [1, NCONST]]))
    for n in range(N):
        nc.sync.dma_start(out=vst[24 * n:24 * n + RH, 0:1], in_=vld)

    def c1(j, lo, hi):
        return cs[lo:hi, j:j + 1]

    # ---------------- phase 0: pools + collective ----------------
    for n in range(N):
        for kc in range(KC):
            i = n * KC + kc
            nc.vector.reduce_sum(out=pool_p[:, kc * N + n: kc * N + n + 1],
                                 in_=ft[:, i, OWN0:OWN0 + R * W], axis=AX.X)
            nc.vector.tensor_reduce(out=pool_p[:, 10 + kc * N + n: 11 + kc * N + n],
                                    in_=ft[:, i, OWN0:OWN0 + R * W], axis=AX.X,
                                    op=OP.max)
    with tc.tile_pool(name="ccd", bufs=1, space="DRAM") as ccd:
        cc_in = ccd.tile([128, 20], f32)
        cc_out = ccd.tile([NCORES * 128, 20], f32, addr_space="Shared")
        nc.sync.dma_start(out=cc_in, in_=pool_p)
        nc.gpsimd.collective_compute(
            "AllGather", OP.bypass,
            replica_groups=[list(range(NCORES))],
            ins=[cc_in.opt()], outs=[cc_out.opt()])
        gath = _tc_tile(tc, [128, NCORES, 20], f32, name="gath")
        nc.sync.dma_start(out=gath, in_=bass.AP(
            cc_out.tensor, cc_out.offset, [[20, 128], [128 * 20, NCORES], [1, 20]]))
    # reduce over the 8 gathered shards
    nc.vector.reduce_sum(out=gsum, in_=bass.AP(
        gath.tensor, gath.offset, [list(gath.ap[0]), [1, 10], [20, NCORES]]),
        axis=AX.X)
    nc.vector.tensor_reduce(out=gmax, in_=bass.AP(
        gath.tensor, gath.offset + 10, [list(gath.ap[0]), [1, 10], [20, NCORES]]),
        axis=AX.X, op=OP.max)

    # ---------------- phase 0b: channel maps + convs (fp32, DVE) ----------
    from concourse import bass_isa
    with tc.tile_pool(name="tree", bufs=1) as tree, \
         tc.tile_pool(name="csps", bufs=2, space="PSUM") as csps:
        for n in range(N):
            # channel sums on the (otherwise idle) TensorEngine
            csrow = tree.tile([1, SFULL], f32, tag="csrow")
            for ch in range(SFULL // TS):
                cps = csps.tile([1, TS], f32, tag="csum")
                for kc in range(KC):
                    nc.tensor.matmul(cps, ones[:, 0:1],
                                     ft[:, n * KC + kc, ch * TS:(ch + 1) * TS],
                                     start=(kc == 0), stop=(kc == 1))
                nc.scalar.activation(out=csrow[0:1, ch * TS:(ch + 1) * TS],
                                     in_=cps, func=AF.Copy)
            nc.sync.dma_start(
                out=cmsum[24 * n:24 * n + RH, 1:129],
                in_=csrow[0:1, 0:SFULL])
            # channel maxes via gpsimd partition all-reduce
            tmp = tree.tile([128, SFULL], f32, tag="tmp")
            nc.vector.tensor_tensor(out=tmp, in0=ft[:, n * KC, :],
                                    in1=ft[:, n * KC + 1, :], op=OP.max)
            nc.gpsimd.partition_all_reduce(out_ap=tmp[:], in_ap=tmp[:],
                                           channels=128,
                                           reduce_op=bass_isa.ReduceOp.max)
            nc.sync.dma_start(
                out=cmmax[24 * n:24 * n + RH, 1:129],
                in_=tmp[0:1, 0:SFULL])

    cscr = _tc_tile(tc, [120, CW], f32, name="cscr")
    nc.vector.memset(cscr, 0.0)

    def conv3(dst, srcs_taps):
        """dst[0:120,1:129] = sum over (src, taps9): taps[3dy+dx]*src[p+dy-1, dx:dx+128].
        Row (partition) shifts go through a DMA copy; every compute op is
        base-partition-0 over all 120 rows (junk rows are never consumed)."""
        first = True
        for src, taps in srcs_taps:
            for dy in range(3):
                if dy == 1:
                    sb = src
                else:
                    if dy == 0:
                        nc.sync.dma_start(out=cscr[1:120, :], in_=src[0:119, :])
                    else:
                        nc.sync.dma_start(out=cscr[0:119, :], in_=src[1:120, :])
                    sb = cscr
                for dx in range(3):
                    tap = taps[3 * dy + dx]
                    in0 = sb[0:120, dx:dx + 128]
                    o = dst[0:120, 1:129]
                    if isinstance(tap, (float, int)):
                        s = float(tap)
                    else:
                        s = c1(tap, 0, 120)
                    if first:
                        nc.vector.tensor_scalar(out=o, in0=in0, scalar1=s,
                                                scalar2=None, op0=OP.mult)
                        first = False
                    else:
                        nc.vector.scalar_tensor_tensor(
                            out=o, in0=in0, scalar=s, in1=o,
                            op0=OP.mult, op1=OP.add)

    # sp_att = sigmoid(conv([mean, max]))
    conv3(spatt, [(cmsum, [CI_SPW0 + j for j in range(9)]),
                  (cmmax, [CI_SPW1 + j for j in range(9)])])
    nc.scalar.activation(out=spatt[0:120, 1:129], in_=spatt[0:120, 1:129],
                         func=AF.Sigmoid)
    # act = conv(sigmoid(mean/256) * valid, g2)
    nc.scalar.activation(out=sigm[0:120, 1:129], in_=cmsum[0:120, 1:129],
                         func=AF.Sigmoid, scale=1.0 / 256.0)
    nc.vector.tensor_scalar(out=sigm[0:120, 1:129], in0=sigm[0:120, 1:129],
                            scalar1=vst[0:120, 0:1], scalar2=None, op0=OP.mult)
    conv3(actm, [(sigm, [float(G2[dy, dx]) for dy in range(3) for dx in range(3)])])
    # sp_coef = conv(sigmoid(wa*ego + wb*sp_att[1:]) * valid, g2)
    nc.vector.tensor_scalar(out=ebuf[0:20, 1:129], in0=spatt[0:20, 1:129],
                            scalar1=c1(CI_NEGWA, 0, 20), scalar2=c1(CI_WA, 0, 20),
                            op0=OP.mult, op1=OP.add)
    for m in range(1, N):
        nc.sync.dma_start(out=spcin[24 * m:24 * m + 20, 1:129],
                          in_=ebuf[0:20, 1:129])
    nc.vector.scalar_tensor_tensor(
        out=spcin[0:120, 1:129], in0=spatt[0:120, 1:129],
        scalar=c1(CI_WB, 0, 120), in1=spcin[0:120, 1:129],
        op0=OP.mult, op1=OP.add)
    nc.scalar.activation(out=spcin[0:120, 1:129], in_=spcin[0:120, 1:129],
                         func=AF.Sigmoid)
    nc.vector.tensor_scalar(out=spcin[0:120, 1:129], in0=spcin[0:120, 1:129],
                            scalar1=vst[0:120, 0:1], scalar2=None, op0=OP.mult)
    conv3(spcoef, [(spcin, [float(G2[dy, dx]) for dy in range(3) for dx in range(3)])])
    nc.vector.tensor_tensor(out=spact[0:120, 1:129], in0=spcoef[0:120, 1:129],
                            in1=actm[0:120, 1:129], op=OP.mult)
    # flatten spact rows into free dim (single partition) for later broadcast
    spact_bf = _tc_tile(tc, [120, CW], bf16, name="spact_bf")
    nc.vector.tensor_copy(out=spact_bf[0:120, :], in_=spact[0:120, :])
    _saflcm = tc.tile_pool(name="safld", bufs=1, space="DRAM")
    saflpool = _saflcm.__enter__()
    safl = saflpool.tile([4, R * W], bf16)
    for m in range(4):
        nc.sync.dma_start(out=safl[m:m + 1, :],
                          in_=spact_bf[24 * (m + 1) + 2:24 * (m + 1) + 18, 1:129])

    # ---------------- phase 0c: MLP -> ch_att -> ch_coef ----------------
    with tc.tile_pool(name="p0ps", bufs=2, space="PSUM") as p0ps, \
         tc.tile_pool(name="p0sb", bufs=2) as p0sb:
        nc.scalar.mul(out=avs, in_=gsum, mul=1.0 / float(H * W))
        hidps = p0ps.tile([16, N], f32, tag="hid")
        for kc in range(KC):
            nc.tensor.matmul(hidps, m1s[:, kc, :], avs[:, kc * N:(kc + 1) * N],
                             start=(kc == 0), stop=(kc == 1))
        hida = p0sb.tile([16, N], f32, tag="hid_sb")
        nc.scalar.activation(out=hida, in_=hidps, func=AF.Relu)
        hidps2 = p0ps.tile([16, N], f32, tag="hid")
        for kc in range(KC):
            nc.tensor.matmul(hidps2, m1s[:, kc, :], gmax[:, kc * N:(kc + 1) * N],
                             start=(kc == 0), stop=(kc == 1))
        hidm = p0sb.tile([16, N], f32, tag="hid_sb")
        nc.scalar.activation(out=hidm, in_=hidps2, func=AF.Relu)
        for kco in range(KC):
            chps = p0ps.tile([128, N], f32, tag="chps")
            nc.tensor.matmul(chps, m2s[:, kco * 128:(kco + 1) * 128], hida,
                             start=True, stop=False)
            nc.tensor.matmul(chps, m2s[:, kco * 128:(kco + 1) * 128], hidm,
                             start=False, stop=True)
            nc.scalar.activation(out=chatt[:, kco * N:(kco + 1) * N], in_=chps,
                                 func=AF.Sigmoid)
        for kc in range(KC):
            nc.vector.tensor_scalar(out=ego4[:, kc:kc + 1],
                                    in0=chatt[:, kc * N:kc * N + 1],
                                    scalar1=-1.0, scalar2=1.0,
                                    op0=OP.mult, op1=OP.add)
        # ch_coef_pre[m, c] (m=0..3 -> n=1..4)
        chpre = p0ps.tile([4, C], f32, tag="chpre")
        for kc in range(KC):
            nc.vector.tensor_copy(out=ego4m[:, :], in_=_bf(ego4[:, kc:kc + 1], 4))
            nc.tensor.matmul(chpre, ego4m, was[:, kc, :],
                             start=(kc == 0), stop=False)
            nc.tensor.matmul(chpre, chatt[:, kc * N + 1:kc * N + N],
                             wbs[:, kc, :], start=False, stop=(kc == 1))
        nc.vector.memset(chpad, 0.0)
        nc.scalar.activation(out=chpad[:, 1:C + 1], in_=chpre, func=AF.Sigmoid)
        # 3-tap gaussian smoothing along C
        nc.vector.tensor_scalar(out=chc4, in0=chpad[:, 1:C + 1],
                                scalar1=float(G1[1]), scalar2=None, op0=OP.mult)
        nc.vector.scalar_tensor_tensor(out=chc4, in0=chpad[:, 0:C],
                                       scalar=float(G1[0]), in1=chc4,
                                       op0=OP.mult, op1=OP.add)
        nc.vector.scalar_tensor_tensor(out=chc4, in0=chpad[:, 2:C + 2],
                                       scalar=float(G1[2]), in1=chc4,
                                       op0=OP.mult, op1=OP.add)
        for kc in range(KC):
            tp = p0ps.tile([128, 4], f32, tag="tp")
            nc.tensor.transpose(tp, chc4[:, kc * 128:(kc + 1) * 128], id4)
            nc.scalar.activation(out=chcoef[:, kc * 4:(kc + 1) * 4], in_=tp,
                                 func=AF.Copy)

    # ---------------- main loop ----------------
    with tc.tile_pool(name="fbf", bufs=10) as fbfp, \
         tc.tile_pool(name="spb", bufs=3) as spbp, \
         tc.tile_pool(name="mstage", bufs=2) as mst, \
         tc.tile_pool(name="h1p", bufs=2) as h1p, \
         tc.tile_pool(name="h2p", bufs=2) as h2p, \
         tc.tile_pool(name="bcp", bufs=2) as bcp, \
         tc.tile_pool(name="psA", bufs=3, space="PSUM") as psA, \
         tc.tile_pool(name="psB", bufs=2, space="PSUM") as psB, \
         tc.tile_pool(name="psT", bufs=3, space="PSUM") as psT:

        fbf = {}

        def cast_feat(n, eng):
            for t in range(NT):
                tl = fbfp.tile([128, KC, TS], FP8, tag="fbf")
                for kc in range(KC):
                    eng.tensor_copy(
                        out=tl[:, kc, :],
                        in_=ft[:, n * KC + kc, OWN0 + t * TS:OWN0 + (t + 1) * TS])
                fbf[(n, t)] = tl

        cast_feat(0, nc.vector)
        cast_feat(1, nc.vector)

        for n in range(N):
            nxt = (n + 1) % N
            if 1 <= n <= N - 2:
                cast_feat(n + 1, nc.vector)
            elif n == N - 1:
                cast_feat(0, nc.vector)  # gen-0 tiles were recycled
            for t in range(NT):
                # ---- mask / sparse for this (n, t) ----
                if n == 0:
                    spbf = fbf[(0, t)]
                    for kc in range(KC):
                        nc.sync.dma_start(
                            out=sf[0, kc * 128:(kc + 1) * 128, 4 * t:4 * t + 4, :],
                            in_=ft[:, kc, OWN0 + t * TS:OWN0 + (t + 1) * TS])
                        nc.sync.dma_start(
                            out=sm[0, kc * 128:(kc + 1) * 128, 4 * t:4 * t + 4, :],
                            in_=ones)
                else:
                    m = n - 1
                    sab = bcp.tile([128, TS], bf16, tag="sab")
                    nc.sync.dma_start(out=sab, in_=bass.AP(
                        safl.tensor, safl.offset + m * (R * W) + t * TS,
                        [[0, 128], [1, TS]]))
                    spbf = spbp.tile([128, KC, TS], FP8, tag="spb")
                    for kc in range(KC):
                        mtile = mst.tile([128, TS], f32, tag="msk")
                        nc.vector.tensor_scalar(
                            out=mtile, in0=sab,
                            scalar1=chcoef[:, kc * 4 + m:kc * 4 + m + 1],
                            scalar2=THRE, op0=OP.mult, op1=OP.is_gt)
                        nc.vector.reduce_sum(
                            out=mks[:, m * KC * NT + kc * NT + t:
                                    m * KC * NT + kc * NT + t + 1],
                            in_=mtile, axis=AX.X)
                        nc.sync.dma_start(
                            out=sm[n, kc * 128:(kc + 1) * 128, 4 * t:4 * t + 4, :],
                            in_=mtile)
                        spf = mst.tile([128, TS], f32, tag="spf")
                        nc.vector.tensor_tensor(
                            out=spf, in0=mtile,
                            in1=ft[:, n * KC + kc, OWN0 + t * TS:OWN0 + (t + 1) * TS],
                            op=OP.mult)
                        nc.sync.dma_start(
                            out=sf[n, kc * 128:(kc + 1) * 128, 4 * t:4 * t + 4, :],
                            in_=spf)
                        nc.vector.tensor_copy(out=spbf[:, kc, :], in_=spf)

                # ---- layer 1 (pair L=T[n], R=T'[n]) ----
                h1L = h1p.tile([128, OB, TS], FP8, tag="h1L")
                h1R = h1p.tile([128, OB, TS], FP8, tag="h1R")
                for ob in range(OB):
                    oc = slice(ob * 128, (ob + 1) * 128)
                    zL = psA.tile([128, TS], f32, tag="za")
                    zR = psA.tile([128, TS], f32, tag="za")
                    rhsL = [fbf[(n, t)], spbf]
                    rhsR = [fbf[(nxt, t)], spbf]
                    for j in range(2):
                        nc.tensor.matmul(zL, w1s[:, j, :, oc],
                                         rhsL[j][:, 0:KC, :],
                                         start=(j == 0), stop=(j == 1),
                                         perf_mode=mybir.MatmulPerfMode.DoubleRow)
                        nc.tensor.matmul(zR, w1s[:, j, :, oc],
                                         rhsR[j][:, 0:KC, :],
                                         start=(j == 0), stop=(j == 1),
                                         perf_mode=mybir.MatmulPerfMode.DoubleRow)
                    nc.scalar.activation(out=h1L[:, ob, :], in_=zL, func=AF.Relu,
                                         bias=b1s[:, ob:ob + 1])
                    nc.vector.tensor_scalar(out=h1R[:, ob, :], in0=zR,
                                            scalar1=b1s[:, ob:ob + 1],
                                            scalar2=0.0, op0=OP.add, op1=OP.max)

                # ---- layer 2 + layer 3 ----
                TL = psT.tile([1, TS], f32, tag="T")
                TR = psT.tile([1, TS], f32, tag="T")
                for ob in range(OB):
                    oc = slice(ob * 128, (ob + 1) * 128)
                    z2L = psB.tile([128, TS], f32, tag="zb")
                    z2R = psB.tile([128, TS], f32, tag="zb")
                    for j in range(4):
                        nc.tensor.matmul(z2L, w2s[:, j, :, oc],
                                         h1L[:, 2 * j:2 * j + 2, :],
                                         start=(j == 0), stop=(j == 3),
                                         perf_mode=mybir.MatmulPerfMode.DoubleRow)
                        nc.tensor.matmul(z2R, w2s[:, j, :, oc],
                                         h1R[:, 2 * j:2 * j + 2, :],
                                         start=(j == 0), stop=(j == 3),
                                         perf_mode=mybir.MatmulPerfMode.DoubleRow)
                    h2L = h2p.tile([128, TS], bf16, tag="h2L")
                    nc.scalar.activation(out=h2L, in_=z2L, func=AF.Relu,
                                         bias=b2s[:, ob:ob + 1])
                    h2R = h2p.tile([128, TS], bf16, tag="h2R")
                    nc.vector.tensor_scalar(out=h2R, in0=z2R,
                                            scalar1=b2s[:, ob:ob + 1],
                                            scalar2=0.0, op0=OP.add, op1=OP.max)
                    nc.tensor.matmul(TL, w3s[:, ob, :], h2L,
                                     start=(ob == 0), stop=(ob == 7))
                    nc.tensor.matmul(TR, w3s[:, ob, :], h2R,
                                     start=(ob == 0), stop=(ob == 7))
                # softplus partials
                idx = n * NT + t
                # softplus(x) = ln(1 + e^x); x = -(T+b3) for joint, +(T'+b3)
                nc.scalar.activation(out=spscr, in_=TL, func=AF.Exp,
                                     scale=-1.0, bias=c1(CI_NEGB3, 0, 1))
                nc.scalar.activation(out=spscr2, in_=TR, func=AF.Exp,
                                     scale=1.0, bias=c1(CI_B3, 0, 1))
                nc.scalar.activation(out=spscr, in_=spscr, func=AF.Ln,
                                     bias=1.0, accum_out=jacc[0:1, idx:idx + 1])
                nc.scalar.activation(out=spscr2, in_=spscr2, func=AF.Ln,
                                     bias=1.0, accum_out=macc[0:1, idx:idx + 1])

        # ---- final scalar partials ----
        nc.vector.reduce_sum(out=statsb[0:1, 0:1], in_=jacc, axis=AX.X)
        nc.vector.reduce_sum(out=statsb[0:1, 1:2], in_=macc, axis=AX.X)
        mkcol = _tc_tile(tc, [128, 1], f32, name="mkcol")
        nc.vector.reduce_sum(out=mkcol, in_=mks, axis=AX.X)
        stps = psT.tile([1, TS], f32, tag="T")
        nc.tensor.matmul(stps[0:1, 0:1], ones[:, 0:1], mkcol,
                         start=True, stop=True)
        nc.scalar.activation(out=statsb[0:1, 2:3], in_=stps[0:1, 0:1],
                             func=AF.Copy)
        nc.sync.dma_start(out=stats, in_=statsb)


def _prep_inputs(inputs):
    f32 = np.float32
    bf = ml_dtypes.bfloat16
    feat = np.ascontiguousarray(inputs["feat"], dtype=f32)
    padded = np.zeros((N, C, H + 4, W), f32)
    padded[:, :, 2:H + 2, :] = feat
    st_w1 = np.asarray(inputs["st_w1"], f32)
    st_w2 = np.asarray(inputs["st_w2"], f32)
    st_w3 = np.asarray(inputs["st_w3"], f32)
    ch_fus_w = np.asarray(inputs["ch_fus_w"], f32)
    sp_req_w = np.asarray(inputs["sp_req_w"], f32)
    sp_fus_w = np.asarray(inputs["sp_fus_w"], f32)
    cstv = np.zeros((NCONST,), f32)
    cstv[CI_SPW0:CI_SPW0 + 9] = sp_req_w[0, 0].reshape(-1) * f32(1.0 / 256.0)
    cstv[CI_SPW1:CI_SPW1 + 9] = sp_req_w[0, 1].reshape(-1)
    cstv[CI_WA] = sp_fus_w[0, 0]
    cstv[CI_NEGWA] = -sp_fus_w[0, 0]
    cstv[CI_WB] = sp_fus_w[0, 1]
    cstv[CI_B3] = np.asarray(inputs["st_b3"], f32)[0]
    cstv[CI_NEGB3] = -cstv[CI_B3]
    shared = {
        "w1t": np.ascontiguousarray(
            st_w1.T.reshape(2, 2, 128, 1024).transpose(2, 0, 1, 3)
        ).astype(ml_dtypes.float8_e4m3),
        "w2t": np.ascontiguousarray(
            st_w2.T.reshape(4, 2, 128, 1024).transpose(2, 0, 1, 3)
        ).astype(ml_dtypes.float8_e4m3),
        "w3t": np.ascontiguousarray(st_w3.T).astype(bf),
        "b1m": np.ascontiguousarray(np.asarray(inputs["st_b1"], f32).reshape(OB, 128).T),
        "b2m": np.ascontiguousarray(np.asarray(inputs["st_b2"], f32).reshape(OB, 128).T),
        "m1t": np.ascontiguousarray(np.asarray(inputs["mlp_w1"], f32).T),
        "m2t": np.ascontiguousarray(np.asarray(inputs["mlp_w2"], f32).T),
        "wat": np.ascontiguousarray(ch_fus_w[:, :C].T),
        "wbt": np.ascontiguousarray(ch_fus_w[:, C:].T),
        "cst": cstv,
    }
    in_maps = []
    for i in range(NCORES):
        vldv = np.zeros((RH, 1), f32)
        for r in range(RH):
            g = 16 * i + r - 2
            vldv[r, 0] = 1.0 if 0 <= g < H else 0.0
        m = dict(shared)
        m["feat"] = np.ascontiguousarray(padded[:, :, 16 * i:16 * i + RH, :])
        m["vld"] = vldv
        in_maps.append(m)
    return in_maps


def kernel(**inputs):
    if "nc" not in _CACHED:
        _CACHED["nc"] = build()
    nc = _CACHED["nc"]
    in_maps = _prep_inputs(inputs)
    res = run_bass_kernel_spmd(nc, in_maps, core_ids=list(range(NCORES)),
                               **_CACHED.get("run_kwargs", {}))
    _CACHED["last_result"] = res
    sparse_feature = np.empty((N, C, H, W), np.float32)
    sparse_mask = np.empty((N, C, H, W), np.float32)
    jsum = msum = mksum = 0.0
    for i in range(NCORES):
        r = res.results[i]
        sparse_feature[:, :, 16 * i:16 * (i + 1), :] = r["sf"]
        sparse_mask[:, :, 16 * i:16 * (i + 1), :] = r["sm"]
        jsum += float(r["stats"][0, 0])
        msum += float(r["stats"][0, 1])
        mksum += float(r["stats"][0, 2])
    npix = float(N * H * W)
    total_loss = np.float32(jsum / npix + msum / npix)
    mean_rate = np.float32(mksum / float((N - 1) * C * H * W))
    return (sparse_feature, total_loss, mean_rate, sparse_mask)


# revision 22
# speedup vs baseline: 1.6711x; 1.0192x over previous
"""Trainium2 Bass kernel for nn_Communication (gnn_message_passing).

Sharding: H=128 rows split 16/core across 8 NeuronCores; every core handles all
N=5 batch elements and all C=256 channels for its row slab. Each core receives
a 20-row slice (2-row halos, zero rows at the global image edges) so the SPMD
program is identical on every core. The only cross-core communication is one
AllGather of the per-core [128,20] channel-pool partials (avg-sums and maxes);
the scalar loss/mean-rate reductions return per-core partials that the host
sums.

Precision: attention/mask path in fp32; the statistics network (two 3-layer
pointwise MLPs, ~87% of FLOPs) runs bf16 on the TensorEngine with fp32 PSUM
accumulation. T and T' share feat/sparse bf16 tiles per (n, s-tile).
"""
import numpy as np
import ml_dtypes

import concourse.bass as bass
import concourse.tile as tile
import concourse.mybir as mybir
from concourse import bacc
from concourse.bass_utils import run_bass_kernel_spmd
from concourse.masks import make_identity

NCORES = 8
N, C, H, W = 5, 256, 128, 128
R = H // NCORES          # owned rows per core (16)
RH = R + 4               # rows incl 2-row halo (20)
KC = C // 128            # channel chunks (2)
OB = 8                   # 1024 hidden / 128
TS = 512                 # free-dim tile (4 rows x 128)
NT = (R * W) // TS       # s-tiles per core (4)
SFULL = RH * W           # 2560 (free size incl halo)
OWN0 = 2 * W             # first owned col (rows 2..18 of 20)
THRE = 0.01
F32 = mybir.dt.float32
BF16 = mybir.dt.bfloat16
FP8 = mybir.dt.float8e4
AX = mybir.AxisListType
OP = mybir.AluOpType
AF = mybir.ActivationFunctionType

# gaussian taps (compile-time constants, replicated from reference)
_c = 3 // 2
_x, _y = np.mgrid[-_c:3 - _c, -_c:3 - _c]
G2 = (1.0 / (2.0 * np.pi) * np.exp(-(_x ** 2 + _y ** 2) / 2.0)).astype(np.float32)
_g1x = np.arange(-1, 2, dtype=np.float32)
_g1 = np.exp(-_g1x ** 2 / 2.0)
G1 = (_g1 / _g1.sum()).astype(np.float32)

# consts vector indices
CI_SPW0 = 0     # 9 taps, in-ch 0 (pre-scaled by 1/256)
CI_SPW1 = 9     # 9 taps, in-ch 1
CI_WA = 18
CI_NEGWA = 19
CI_WB = 20
CI_B3 = 21
CI_NEGB3 = 22
NCONST = 24

_CACHED = {}


def _bf(ap2d, nrep):
    """Free-dim broadcast: [P,1] AP -> [P,nrep] via 0-stride."""
    return bass.AP(ap2d.tensor, ap2d.offset, [list(ap2d.ap[0]), [0, nrep]])


ACT_TABLE_PATCH = True


def _patch_act_tables():
    """Order activation tables so one table (Relu+Exp+Ln+Copy) serves the
    whole main loop -- avoids per-tile ACT table reloads."""
    import concourse.bacc as _bm
    if not ACT_TABLE_PATCH or getattr(_bm, "_act_tbl_patched", False):
        return
    _orig = _bm.get_activation_tables

    def _reordered(arch):
        t = _orig(arch)
        pref = 'natural_log_exp_and_others'
        order = [pref] + [k for k in t if k != pref]
        return {k: t[k] for k in order}

    _bm.get_activation_tables = _reordered
    _bm._act_tbl_patched = True


def build():
    _patch_act_tables()
    nc = bacc.Bacc("TRN2", target_bir_lowering=False, debug=False,
                   num_devices=NCORES)
    feat = nc.dram_tensor("feat", [N, C, RH, W], F32, kind="ExternalInput").ap()
    w1t = nc.dram_tensor("w1t", [128, 2, 2, 1024], FP8, kind="ExternalInput").ap()
    w2t = nc.dram_tensor("w2t", [128, 4, 2, 1024], FP8, kind="ExternalInput").ap()
    w3t = nc.dram_tensor("w3t", [1024, 1], BF16, kind="ExternalInput").ap()
    b1m = nc.dram_tensor("b1m", [128, OB], F32, kind="ExternalInput").ap()
    b2m = nc.dram_tensor("b2m", [128, OB], F32, kind="ExternalInput").ap()
    m1t = nc.dram_tensor("m1t", [C, 16], F32, kind="ExternalInput").ap()
    m2t = nc.dram_tensor("m2t", [16, C], F32, kind="ExternalInput").ap()
    wat = nc.dram_tensor("wat", [C, C], F32, kind="ExternalInput").ap()
    wbt = nc.dram_tensor("wbt", [C, C], F32, kind="ExternalInput").ap()
    cst = nc.dram_tensor("cst", [NCONST], F32, kind="ExternalInput").ap()
    vld = nc.dram_tensor("vld", [RH, 1], F32, kind="ExternalInput").ap()

    sf = nc.dram_tensor("sf", [N, C, R, W], F32, kind="ExternalOutput").ap()
    sm = nc.dram_tensor("sm", [N, C, R, W], F32, kind="ExternalOutput").ap()
    stats = nc.dram_tensor("stats", [1, 4], F32, kind="ExternalOutput").ap()

    with tile.TileContext(nc) as tc:
        _build_body(nc, tc, feat, w1t, w2t, w3t, b1m, b2m, m1t, m2t, wat, wbt,
                    cst, vld, sf, sm, stats)
    nc.compile()
    return nc




def _build_body(nc, tc, feat, w1t, w2t, w3t, b1m, b2m, m1t, m2t, wat, wbt,
                cst, vld, sf, sm, stats):
    f32, bf16 = F32, BF16
    respool = tc.tile_pool(name="res", bufs=1)
    res = respool.__enter__()

    def _tc_tile(tc, shape, dtype, name):
        return res.tile(shape, dtype, tag=name, name=name)

    # ---------------- persistent tiles ----------------
    ft = _tc_tile(tc, [128, 2 * N, SFULL], f32, name="ft")          # resident feat
    w1s = _tc_tile(tc, [128, 2, 2, 1024], FP8, name="w1s")
    w2s = _tc_tile(tc, [128, 4, 2, 1024], FP8, name="w2s")
    w3s = _tc_tile(tc, [128, OB, 1], bf16, name="w3s")
    b1s = _tc_tile(tc, [128, OB], f32, name="b1s")
    b2s = _tc_tile(tc, [128, OB], f32, name="b2s")
    m1s = _tc_tile(tc, [128, KC, 16], f32, name="m1s")
    m2s = _tc_tile(tc, [16, C], f32, name="m2s")
    was = _tc_tile(tc, [128, KC, C], f32, name="was")
    wbs = _tc_tile(tc, [128, KC, C], f32, name="wbs")
    cs = _tc_tile(tc, [128, NCONST], f32, name="cs")
    vst = _tc_tile(tc, [120, 1], f32, name="vst")
    ones = _tc_tile(tc, [128, TS], f32, name="ones")
    id4 = _tc_tile(tc, [4, 4], f32, name="id4")

    # conv buffers: 5 row-blocks of 24 partitions; data rows r=0..19 at
    # partition 24n+r, cols 1..128 hold W, col 0/129 stay zero.
    CW = 132
    cmsum = _tc_tile(tc, [120, CW], f32, name="cmsum")
    cmmax = _tc_tile(tc, [120, CW], f32, name="cmmax")
    sigm = _tc_tile(tc, [120, CW], f32, name="sigm")
    spatt = _tc_tile(tc, [120, CW], f32, name="spatt")
    ebuf = _tc_tile(tc, [24, CW], f32, name="ebuf")
    spcin = _tc_tile(tc, [120, CW], f32, name="spcin")
    spcoef = _tc_tile(tc, [120, CW], f32, name="spcoef")
    actm = _tc_tile(tc, [120, CW], f32, name="actm")
    spact = _tc_tile(tc, [120, CW], f32, name="spact")

    pool_p = _tc_tile(tc, [128, 20], f32, name="pool_p")            # local partials
    gsum = _tc_tile(tc, [128, 10], f32, name="gsum")
    gmax = _tc_tile(tc, [128, 10], f32, name="gmax")
    avs = _tc_tile(tc, [128, 10], f32, name="avs")
    chatt = _tc_tile(tc, [128, 10], f32, name="chatt")
    ego4 = _tc_tile(tc, [128, KC], f32, name="ego4")
    ego4m = _tc_tile(tc, [128, 4], f32, name="ego4m")
    chpad = _tc_tile(tc, [4, C + 2], f32, name="chpad")
    chc4 = _tc_tile(tc, [4, C], f32, name="chc4")
    chcoef = _tc_tile(tc, [128, 2 * 4], f32, name="chcoef")         # col kc*4+m
    jacc = _tc_tile(tc, [1, N * NT], f32, name="jacc")
    macc = _tc_tile(tc, [1, N * NT], f32, name="macc")
    mks = _tc_tile(tc, [128, 4 * KC * NT], f32, name="mks")
    statsb = _tc_tile(tc, [1, 4], f32, name="statsb")
    spscr = _tc_tile(tc, [1, TS], f32, name="spscr")
    spscr2 = _tc_tile(tc, [1, TS], f32, name="spscr2")

    for t_ in (cmsum, cmmax, sigm, spatt, spcin, spcoef, actm, spact, ebuf,
               vst, statsb):
        nc.vector.memset(t_, 0.0)
    nc.vector.memset(ones, 1.0)
    make_identity(nc, id4)

    # ---------------- input DMAs ----------------
    for n in range(N):
        for kc in range(KC):
            nc.sync.dma_start(
                out=ft[:, n * KC + kc, :],
                in_=feat[n, kc * 128:(kc + 1) * 128, :, :])
    nc.sync.dma_start(out=w1s, in_=w1t)
    nc.sync.dma_start(out=w2s, in_=w2t)
    nc.sync.dma_start(out=w3s, in_=bass.AP(
        w3t.tensor, 0, [[1, 128], [128, OB], [1, 1]]))
    nc.sync.dma_start(out=b1s, in_=b1m)
    nc.sync.dma_start(out=b2s, in_=b2m)
    nc.sync.dma_start(out=m1s, in_=bass.AP(
        m1t.tensor, 0, [[16, 128], [128 * 16, KC], [1, 16]]))
    nc.sync.dma_start(out=m2s, in_=m2t)
    nc.sync.dma_start(out=was, in_=bass.AP(
        wat.tensor, 0, [[C, 128], [128 * C, KC], [1, C]]))
    nc.sync.dma_start(out=wbs, in_=bass.AP(
        wbt.tensor, 0, [[C, 128], [128 * C, KC], [1, C]]))
    nc.sync.dma_start(out=cs, in_=bass.AP(
        cst.tensor, 0, [[0, 128], [1, NCONST]]))
    for n in range(N):
        nc.sync.dma_start(out=vst[24 * n:24 * n + RH, 0:1], in_=vld)

    def c1(j, lo, hi):
        return cs[lo:hi, j:j + 1]

    # ---------------- phase 0: pools + collective ----------------
    pscrap = _tc_tile(tc, [128, R * W], f32, name="pscrap")
    for n in range(N):
        for kc in range(KC):
            i = n * KC + kc
            nc.scalar.activation(out=pscrap, in_=ft[:, i, OWN0:OWN0 + R * W],
                                 func=AF.Copy,
                                 accum_out=pool_p[:, kc * N + n: kc * N + n + 1])
            nc.vector.tensor_reduce(out=pool_p[:, 10 + kc * N + n: 11 + kc * N + n],
                                    in_=ft[:, i, OWN0:OWN0 + R * W], axis=AX.X,
                                    op=OP.max)
    with tc.tile_pool(name="ccd", bufs=1, space="DRAM") as ccd:
        cc_in = ccd.tile([128, 20], f32)
        cc_out = ccd.tile([NCORES * 128, 20], f32, addr_space="Shared")
        nc.sync.dma_start(out=cc_in, in_=pool_p)
        nc.gpsimd.collective_compute(
            "AllGather", OP.bypass,
            replica_groups=[list(range(NCORES))],
            ins=[cc_in.opt()], outs=[cc_out.opt()])
        gath = _tc_tile(tc, [128, NCORES, 20], f32, name="gath")
        nc.sync.dma_start(out=gath, in_=bass.AP(
            cc_out.tensor, cc_out.offset, [[20, 128], [128 * 20, NCORES], [1, 20]]))
    # reduce over the 8 gathered shards
    nc.vector.reduce_sum(out=gsum, in_=bass.AP(
        gath.tensor, gath.offset, [list(gath.ap[0]), [1, 10], [20, NCORES]]),
        axis=AX.X)
    nc.vector.tensor_reduce(out=gmax, in_=bass.AP(
        gath.tensor, gath.offset + 10, [list(gath.ap[0]), [1, 10], [20, NCORES]]),
        axis=AX.X, op=OP.max)

    # ---------------- phase 0b: channel maps + convs (fp32, DVE) ----------
    from concourse import bass_isa
    with tc.tile_pool(name="tree", bufs=1) as tree, \
         tc.tile_pool(name="csps", bufs=2, space="PSUM") as csps:
        for n in range(N):
            # channel sums on the (otherwise idle) TensorEngine
            csrow = tree.tile([1, SFULL], f32, tag="csrow")
            for ch in range(SFULL // TS):
                cps = csps.tile([1, TS], f32, tag="csum")
                for kc in range(KC):
                    nc.tensor.matmul(cps, ones[:, 0:1],
                                     ft[:, n * KC + kc, ch * TS:(ch + 1) * TS],
                                     start=(kc == 0), stop=(kc == 1))
                nc.scalar.activation(out=csrow[0:1, ch * TS:(ch + 1) * TS],
                                     in_=cps, func=AF.Copy)
            nc.sync.dma_start(
                out=cmsum[24 * n:24 * n + RH, 1:129],
                in_=csrow[0:1, 0:SFULL])
            # channel maxes via gpsimd partition all-reduce
            tmp = tree.tile([128, SFULL], f32, tag="tmp")
            nc.vector.tensor_tensor(out=tmp, in0=ft[:, n * KC, :],
                                    in1=ft[:, n * KC + 1, :], op=OP.max)
            nc.gpsimd.partition_all_reduce(out_ap=tmp[:], in_ap=tmp[:],
                                           channels=128,
                                           reduce_op=bass_isa.ReduceOp.max)
            nc.sync.dma_start(
                out=cmmax[24 * n:24 * n + RH, 1:129],
                in_=tmp[0:1, 0:SFULL])

    cscr = _tc_tile(tc, [120, CW], f32, name="cscr")
    nc.vector.memset(cscr, 0.0)

    def conv3(dst, srcs_taps):
        """dst[0:120,1:129] = sum over (src, taps9): taps[3dy+dx]*src[p+dy-1, dx:dx+128].
        Row (partition) shifts go through a DMA copy; every compute op is
        base-partition-0 over all 120 rows (junk rows are never consumed)."""
        first = True
        for src, taps in srcs_taps:
            for dy in range(3):
                if dy == 1:
                    sb = src
                else:
                    if dy == 0:
                        nc.sync.dma_start(out=cscr[1:120, :], in_=src[0:119, :])
                    else:
                        nc.sync.dma_start(out=cscr[0:119, :], in_=src[1:120, :])
                    sb = cscr
                for dx in range(3):
                    tap = taps[3 * dy + dx]
                    in0 = sb[0:120, dx:dx + 128]
                    o = dst[0:120, 1:129]
                    if isinstance(tap, (float, int)):
                        s = float(tap)
                    else:
                        s = c1(tap, 0, 120)
                    if first:
                        nc.vector.tensor_scalar(out=o, in0=in0, scalar1=s,
                                                scalar2=None, op0=OP.mult)
                        first = False
                    else:
                        nc.vector.scalar_tensor_tensor(
                            out=o, in0=in0, scalar=s, in1=o,
                            op0=OP.mult, op1=OP.add)

    # sp_att = sigmoid(conv([mean, max]))
    conv3(spatt, [(cmsum, [CI_SPW0 + j for j in range(9)]),
                  (cmmax, [CI_SPW1 + j for j in range(9)])])
    nc.scalar.activation(out=spatt[0:120, 1:129], in_=spatt[0:120, 1:129],
                         func=AF.Sigmoid)
    # act = conv(sigmoid(mean/256) * valid, g2)
    nc.scalar.activation(out=sigm[0:120, 1:129], in_=cmsum[0:120, 1:129],
                         func=AF.Sigmoid, scale=1.0 / 256.0)
    nc.vector.tensor_scalar(out=sigm[0:120, 1:129], in0=sigm[0:120, 1:129],
                            scalar1=vst[0:120, 0:1], scalar2=None, op0=OP.mult)
    conv3(actm, [(sigm, [float(G2[dy, dx]) for dy in range(3) for dx in range(3)])])
    # sp_coef = conv(sigmoid(wa*ego + wb*sp_att[1:]) * valid, g2)
    nc.vector.tensor_scalar(out=ebuf[0:20, 1:129], in0=spatt[0:20, 1:129],
                            scalar1=c1(CI_NEGWA, 0, 20), scalar2=c1(CI_WA, 0, 20),
                            op0=OP.mult, op1=OP.add)
    for m in range(1, N):
        nc.sync.dma_start(out=spcin[24 * m:24 * m + 20, 1:129],
                          in_=ebuf[0:20, 1:129])
    nc.vector.scalar_tensor_tensor(
        out=spcin[0:120, 1:129], in0=spatt[0:120, 1:129],
        scalar=c1(CI_WB, 0, 120), in1=spcin[0:120, 1:129],
        op0=OP.mult, op1=OP.add)
    nc.scalar.activation(out=spcin[0:120, 1:129], in_=spcin[0:120, 1:129],
                         func=AF.Sigmoid)
    nc.vector.tensor_scalar(out=spcin[0:120, 1:129], in0=spcin[0:120, 1:129],
                            scalar1=vst[0:120, 0:1], scalar2=None, op0=OP.mult)
    conv3(spcoef, [(spcin, [float(G2[dy, dx]) for dy in range(3) for dx in range(3)])])
    nc.vector.tensor_tensor(out=spact[0:120, 1:129], in0=spcoef[0:120, 1:129],
                            in1=actm[0:120, 1:129], op=OP.mult)
    # flatten spact rows into free dim (single partition) for later broadcast
    spact_bf = _tc_tile(tc, [120, CW], bf16, name="spact_bf")
    nc.vector.tensor_copy(out=spact_bf[0:120, :], in_=spact[0:120, :])
    _saflcm = tc.tile_pool(name="safld", bufs=1, space="DRAM")
    saflpool = _saflcm.__enter__()
    safl = saflpool.tile([4, R * W], bf16)
    for m in range(4):
        nc.sync.dma_start(out=safl[m:m + 1, :],
                          in_=spact_bf[24 * (m + 1) + 2:24 * (m + 1) + 18, 1:129])

    # ---------------- phase 0c: MLP -> ch_att -> ch_coef ----------------
    with tc.tile_pool(name="p0ps", bufs=2, space="PSUM") as p0ps, \
         tc.tile_pool(name="p0sb", bufs=2) as p0sb:
        nc.scalar.mul(out=avs, in_=gsum, mul=1.0 / float(H * W))
        hidps = p0ps.tile([16, N], f32, tag="hid")
        for kc in range(KC):
            nc.tensor.matmul(hidps, m1s[:, kc, :], avs[:, kc * N:(kc + 1) * N],
                             start=(kc == 0), stop=(kc == 1))
        hida = p0sb.tile([16, N], f32, tag="hid_sb")
        nc.scalar.activation(out=hida, in_=hidps, func=AF.Relu)
        hidps2 = p0ps.tile([16, N], f32, tag="hid")
        for kc in range(KC):
            nc.tensor.matmul(hidps2, m1s[:, kc, :], gmax[:, kc * N:(kc + 1) * N],
                             start=(kc == 0), stop=(kc == 1))
        hidm = p0sb.tile([16, N], f32, tag="hid_sb")
        nc.scalar.activation(out=hidm, in_=hidps2, func=AF.Relu)
        for kco in range(KC):
            chps = p0ps.tile([128, N], f32, tag="chps")
            nc.tensor.matmul(chps, m2s[:, kco * 128:(kco + 1) * 128], hida,
                             start=True, stop=False)
            nc.tensor.matmul(chps, m2s[:, kco * 128:(kco + 1) * 128], hidm,
                             start=False, stop=True)
            nc.scalar.activation(out=chatt[:, kco * N:(kco + 1) * N], in_=chps,
                                 func=AF.Sigmoid)
        for kc in range(KC):
            nc.vector.tensor_scalar(out=ego4[:, kc:kc + 1],
                                    in0=chatt[:, kc * N:kc * N + 1],
                                    scalar1=-1.0, scalar2=1.0,
                                    op0=OP.mult, op1=OP.add)
        # ch_coef_pre[m, c] (m=0..3 -> n=1..4)
        chpre = p0ps.tile([4, C], f32, tag="chpre")
        for kc in range(KC):
            nc.vector.tensor_copy(out=ego4m[:, :], in_=_bf(ego4[:, kc:kc + 1], 4))
            nc.tensor.matmul(chpre, ego4m, was[:, kc, :],
                             start=(kc == 0), stop=False)
            nc.tensor.matmul(chpre, chatt[:, kc * N + 1:kc * N + N],
                             wbs[:, kc, :], start=False, stop=(kc == 1))
        nc.vector.memset(chpad, 0.0)
        nc.scalar.activation(out=chpad[:, 1:C + 1], in_=chpre, func=AF.Sigmoid)
        # 3-tap gaussian smoothing along C
        nc.vector.tensor_scalar(out=chc4, in0=chpad[:, 1:C + 1],
                                scalar1=float(G1[1]), scalar2=None, op0=OP.mult)
        nc.vector.scalar_tensor_tensor(out=chc4, in0=chpad[:, 0:C],
                                       scalar=float(G1[0]), in1=chc4,
                                       op0=OP.mult, op1=OP.add)
        nc.vector.scalar_tensor_tensor(out=chc4, in0=chpad[:, 2:C + 2],
                                       scalar=float(G1[2]), in1=chc4,
                                       op0=OP.mult, op1=OP.add)
        for kc in range(KC):
            tp = p0ps.tile([128, 4], f32, tag="tp")
            nc.tensor.transpose(tp, chc4[:, kc * 128:(kc + 1) * 128], id4)
            nc.scalar.activation(out=chcoef[:, kc * 4:(kc + 1) * 4], in_=tp,
                                 func=AF.Copy)

    # ---------------- main loop ----------------
    with tc.tile_pool(name="fbf", bufs=10) as fbfp, \
         tc.tile_pool(name="spb", bufs=3) as spbp, \
         tc.tile_pool(name="mstage", bufs=2) as mst, \
         tc.tile_pool(name="h1p", bufs=2) as h1p, \
         tc.tile_pool(name="h2p", bufs=2) as h2p, \
         tc.tile_pool(name="bcp", bufs=2) as bcp, \
         tc.tile_pool(name="psA", bufs=2, space="PSUM") as psA, \
         tc.tile_pool(name="psB", bufs=3, space="PSUM") as psB, \
         tc.tile_pool(name="psT", bufs=3, space="PSUM") as psT:

        fbf = {}

        def cast_feat(n, eng):
            for t in range(NT):
                tl = fbfp.tile([128, KC, TS], FP8, tag="fbf")
                for kc in range(KC):
                    eng.tensor_copy(
                        out=tl[:, kc, :],
                        in_=ft[:, n * KC + kc, OWN0 + t * TS:OWN0 + (t + 1) * TS])
                fbf[(n, t)] = tl

        cast_feat(0, nc.vector)
        cast_feat(1, nc.vector)

        for n in range(N):
            nxt = (n + 1) % N
            if 1 <= n <= N - 2:
                cast_feat(n + 1, nc.vector)
            elif n == N - 1:
                cast_feat(0, nc.vector)  # gen-0 tiles were recycled
            for t in range(NT):
                # ---- mask / sparse for this (n, t) ----
                if n == 0:
                    spbf = fbf[(0, t)]
                    for kc in range(KC):
                        nc.sync.dma_start(
                            out=sf[0, kc * 128:(kc + 1) * 128, 4 * t:4 * t + 4, :],
                            in_=ft[:, kc, OWN0 + t * TS:OWN0 + (t + 1) * TS])
                        nc.sync.dma_start(
                            out=sm[0, kc * 128:(kc + 1) * 128, 4 * t:4 * t + 4, :],
                            in_=ones)
                else:
                    m = n - 1
                    sab = bcp.tile([128, TS], bf16, tag="sab")
                    nc.sync.dma_start(out=sab, in_=bass.AP(
                        safl.tensor, safl.offset + m * (R * W) + t * TS,
                        [[0, 128], [1, TS]]))
                    spbf = spbp.tile([128, KC, TS], FP8, tag="spb")
                    for kc in range(KC):
                        mtile = mst.tile([128, TS], f32, tag="msk")
                        nc.vector.tensor_scalar(
                            out=mtile, in0=sab,
                            scalar1=chcoef[:, kc * 4 + m:kc * 4 + m + 1],
                            scalar2=THRE, op0=OP.mult, op1=OP.is_gt)
                        nc.vector.reduce_sum(
                            out=mks[:, m * KC * NT + kc * NT + t:
                                    m * KC * NT + kc * NT + t + 1],
                            in_=mtile, axis=AX.X)
                        nc.sync.dma_start(
                            out=sm[n, kc * 128:(kc + 1) * 128, 4 * t:4 * t + 4, :],
                            in_=mtile)
                        spf = mst.tile([128, TS], f32, tag="spf")
                        nc.vector.tensor_tensor(
                            out=spf, in0=mtile,
                            in1=ft[:, n * KC + kc, OWN0 + t * TS:OWN0 + (t + 1) * TS],
                            op=OP.mult)
                        nc.sync.dma_start(
                            out=sf[n, kc * 128:(kc + 1) * 128, 4 * t:4 * t + 4, :],
                            in_=spf)
                        nc.vector.tensor_copy(out=spbf[:, kc, :], in_=spf)

                # ---- layer 1 (pair L=T[n], R=T'[n]) ----
                h1L = h1p.tile([128, OB, TS], FP8, tag="h1L")
                h1R = h1p.tile([128, OB, TS], FP8, tag="h1R")
                for ob in range(OB):
                    oc = slice(ob * 128, (ob + 1) * 128)
                    zL = psA.tile([128, TS], f32, tag="za")
                    zR = psA.tile([128, TS], f32, tag="za")
                    rhsL = [fbf[(n, t)], spbf]
                    rhsR = [fbf[(nxt, t)], spbf]
                    for j in range(2):
                        nc.tensor.matmul(zL, w1s[:, j, :, oc],
                                         rhsL[j][:, 0:KC, :],
                                         start=(j == 0), stop=(j == 1),
                                         perf_mode=mybir.MatmulPerfMode.DoubleRow)
                        nc.tensor.matmul(zR, w1s[:, j, :, oc],
                                         rhsR[j][:, 0:KC, :],
                                         start=(j == 0), stop=(j == 1),
                                         perf_mode=mybir.MatmulPerfMode.DoubleRow)
                    nc.scalar.activation(out=h1L[:, ob, :], in_=zL, func=AF.Relu,
                                         bias=b1s[:, ob:ob + 1])
                    nc.vector.tensor_scalar(out=h1R[:, ob, :], in0=zR,
                                            scalar1=b1s[:, ob:ob + 1],
                                            scalar2=0.0, op0=OP.add, op1=OP.max)

                # ---- layer 2 + layer 3 ----
                TL = psT.tile([1, TS], f32, tag="T")
                TR = psT.tile([1, TS], f32, tag="T")
                for ob in range(OB):
                    oc = slice(ob * 128, (ob + 1) * 128)
                    z2L = psB.tile([128, TS], f32, tag="zb")
                    z2R = psB.tile([128, TS], f32, tag="zb")
                    for j in range(4):
                        nc.tensor.matmul(z2L, w2s[:, j, :, oc],
                                         h1L[:, 2 * j:2 * j + 2, :],
                                         start=(j == 0), stop=(j == 3),
                                         perf_mode=mybir.MatmulPerfMode.DoubleRow)
                        nc.tensor.matmul(z2R, w2s[:, j, :, oc],
                                         h1R[:, 2 * j:2 * j + 2, :],
                                         start=(j == 0), stop=(j == 3),
                                         perf_mode=mybir.MatmulPerfMode.DoubleRow)
                    h2L = h2p.tile([128, TS], bf16, tag="h2L")
                    nc.scalar.activation(out=h2L, in_=z2L, func=AF.Relu,
                                         bias=b2s[:, ob:ob + 1])
                    h2R = h2p.tile([128, TS], bf16, tag="h2R")
                    nc.vector.tensor_scalar(out=h2R, in0=z2R,
                                            scalar1=b2s[:, ob:ob + 1],
                                            scalar2=0.0, op0=OP.add, op1=OP.max)
                    nc.tensor.matmul(TL, w3s[:, ob, :], h2L,
                                     start=(ob == 0), stop=(ob == 7))
                    nc.tensor.matmul(TR, w3s[:, ob, :], h2R,
                                     start=(ob == 0), stop=(ob == 7))
                # softplus partials
                idx = n * NT + t
                # softplus(x) = ln(1 + e^x); x = -(T+b3) for joint, +(T'+b3)
                nc.scalar.activation(out=spscr, in_=TL, func=AF.Exp,
                                     scale=-1.0, bias=c1(CI_NEGB3, 0, 1))
                nc.scalar.activation(out=spscr2, in_=TR, func=AF.Exp,
                                     scale=1.0, bias=c1(CI_B3, 0, 1))
                nc.scalar.activation(out=spscr, in_=spscr, func=AF.Ln,
                                     bias=1.0, accum_out=jacc[0:1, idx:idx + 1])
                nc.scalar.activation(out=spscr2, in_=spscr2, func=AF.Ln,
                                     bias=1.0, accum_out=macc[0:1, idx:idx + 1])

        # ---- final scalar partials ----
        nc.vector.reduce_sum(out=statsb[0:1, 0:1], in_=jacc, axis=AX.X)
        nc.vector.reduce_sum(out=statsb[0:1, 1:2], in_=macc, axis=AX.X)
        mkcol = _tc_tile(tc, [128, 1], f32, name="mkcol")
        nc.vector.reduce_sum(out=mkcol, in_=mks, axis=AX.X)
        stps = psT.tile([1, TS], f32, tag="T")
        nc.tensor.matmul(stps[0:1, 0:1], ones[:, 0:1], mkcol,
                         start=True, stop=True)
        nc.scalar.activation(out=statsb[0:1, 2:3], in_=stps[0:1, 0:1],
                             func=AF.Copy)
        nc.sync.dma_start(out=stats, in_=statsb)


def _prep_inputs(inputs):
    f32 = np.float32
    bf = ml_dtypes.bfloat16
    feat = np.ascontiguousarray(inputs["feat"], dtype=f32)
    padded = np.zeros((N, C, H + 4, W), f32)
    padded[:, :, 2:H + 2, :] = feat
    st_w1 = np.asarray(inputs["st_w1"], f32)
    st_w2 = np.asarray(inputs["st_w2"], f32)
    st_w3 = np.asarray(inputs["st_w3"], f32)
    ch_fus_w = np.asarray(inputs["ch_fus_w"], f32)
    sp_req_w = np.asarray(inputs["sp_req_w"], f32)
    sp_fus_w = np.asarray(inputs["sp_fus_w"], f32)
    cstv = np.zeros((NCONST,), f32)
    cstv[CI_SPW0:CI_SPW0 + 9] = sp_req_w[0, 0].reshape(-1) * f32(1.0 / 256.0)
    cstv[CI_SPW1:CI_SPW1 + 9] = sp_req_w[0, 1].reshape(-1)
    cstv[CI_WA] = sp_fus_w[0, 0]
    cstv[CI_NEGWA] = -sp_fus_w[0, 0]
    cstv[CI_WB] = sp_fus_w[0, 1]
    cstv[CI_B3] = np.asarray(inputs["st_b3"], f32)[0]
    cstv[CI_NEGB3] = -cstv[CI_B3]
    shared = {
        "w1t": np.ascontiguousarray(
            st_w1.T.reshape(2, 2, 128, 1024).transpose(2, 0, 1, 3)
        ).astype(ml_dtypes.float8_e4m3),
        "w2t": np.ascontiguousarray(
            st_w2.T.reshape(4, 2, 128, 1024).transpose(2, 0, 1, 3)
        ).astype(ml_dtypes.float8_e4m3),
        "w3t": np.ascontiguousarray(st_w3.T).astype(bf),
        "b1m": np.ascontiguousarray(np.asarray(inputs["st_b1"], f32).reshape(OB, 128).T),
        "b2m": np.ascontiguousarray(np.asarray(inputs["st_b2"], f32).reshape(OB, 128).T),
        "m1t": np.ascontiguousarray(np.asarray(inputs["mlp_w1"], f32).T),
        "m2t": np.ascontiguousarray(np.asarray(inputs["mlp_w2"], f32).T),
        "wat": np.ascontiguousarray(ch_fus_w[:, :C].T),
        "wbt": np.ascontiguousarray(ch_fus_w[:, C:].T),
        "cst": cstv,
    }
    in_maps = []
    for i in range(NCORES):
        vldv = np.zeros((RH, 1), f32)
        for r in range(RH):
            g = 16 * i + r - 2
            vldv[r, 0] = 1.0 if 0 <= g < H else 0.0
        m = dict(shared)
        m["feat"] = np.ascontiguousarray(padded[:, :, 16 * i:16 * i + RH, :])
        m["vld"] = vldv
        in_maps.append(m)
    return in_maps


def kernel(**inputs):
    if "nc" not in _CACHED:
        _CACHED["nc"] = build()
    nc = _CACHED["nc"]
    in_maps = _prep_inputs(inputs)
    res = run_bass_kernel_spmd(nc, in_maps, core_ids=list(range(NCORES)),
                               **_CACHED.get("run_kwargs", {}))
    _CACHED["last_result"] = res
    sparse_feature = np.empty((N, C, H, W), np.float32)
    sparse_mask = np.empty((N, C, H, W), np.float32)
    jsum = msum = mksum = 0.0
    for i in range(NCORES):
        r = res.results[i]
        sparse_feature[:, :, 16 * i:16 * (i + 1), :] = r["sf"]
        sparse_mask[:, :, 16 * i:16 * (i + 1), :] = r["sm"]
        jsum += float(r["stats"][0, 0])
        msum += float(r["stats"][0, 1])
        mksum += float(r["stats"][0, 2])
    npix = float(N * H * W)
    total_loss = np.float32(jsum / npix + msum / npix)
    mean_rate = np.float32(mksum / float((N - 1) * C * H * W))
    return (sparse_feature, total_loss, mean_rate, sparse_mask)
